# revision 11
# baseline (speedup 1.0000x reference)
"""Trainium2 Bass kernel for ConditionalSimNet2 (moe_routing).

Computation (B=128, FEAT_IN=2048, D=1024, N=P=66 conditions):
    x          = image @ W_emb + b_emb                    [B, D]
    masked_rep = einsum('bd,nde->bne', x, W_rep) + b_rep  [B, N, D]
    embed      = mask_table * masked_rep                  [B, N, D]
    att        = softmax(relu(cat_enc@W1+b1)@W2 + b2)     [P, N]
    cond_feat  = einsum('pn,bnd->bpd', att, embed)        [B, P, D]
    out        = concat([cond_feat, broadcast(x)], 1)     [B, P+N, D]

Sharding: expert-parallel over the 66 conditions on 8 cores (9 each,
zero-padded to 72).  Every core computes x and att redundantly (cheap),
runs its 9 grouped GEMMs against its W_rep shard (the dominant HBM
traffic), exchanges embed slices with AllToAll so each core holds all
66 conditions for its 16-row batch shard, reduces with a single-K
matmul and writes its [66, 16*1024] output shard (p-major, bf16); the
host transposes/upcasts and broadcasts feature_x from core 0's x.

Key structural points vs the naive version:
  - mask_table (and the b_rep bias, via a host-side att-weighted
    correction) are folded into W_rep ON THE HOST, so the device never
    touches masks: no DVE broadcast-multiply, no bias matmuls in the
    hot loop.
  - image arrives pre-transposed/pre-cast (imgT bf16) and W_emb
    pre-cast bf16 k-major: the x GEMM is pure bf16 with no device-side
    image transposes.
  - grouped GEMM is k-outer over groups of conditions: the stationary
    operand (xT k-slice) is reused across the group, keeping the PE
    warm and LDWEIGHTS amortized.
  - DMA queues are specialized: sync ring = W_rep weight stream only
    (big FIFO 2 MB transfers at full HBM rate); scalar ring = startup
    loads + a2a sends + output writes; gpsimd = collectives + recv
    loads.  This keeps a blocked collective wait from ever stalling
    the weight stream or the next group's sends.
  - a2a payloads are bf16 (or whatever WDT the embed is cast to) and
    chunked [1,4,4] so the wire chain starts as early as possible.
  - W_rep can optionally ship in fp8e4m3 (scaled by WSCALE, de-scaled
    through the attention matrix) to halve the dominant HBM stream.
"""

import os
import sys

import numpy as np

try:
    import concourse.bass as bass
except ImportError:  # pragma: no cover - fallback when PYTHONPATH is not set
    sys.path.insert(0, "/opt/trn_rl_repo")
    import concourse.bass as bass

import concourse.mybir as mybir
import concourse.tile as tile
from concourse.bass_utils import run_bass_kernel_spmd
from concourse.masks import make_identity

F32 = mybir.dt.float32
BF16 = mybir.dt.bfloat16
FP8 = mybir.dt.float8e4

B = 128          # batch
FI = 2048        # backbone feature dim
D = 1024         # embed dim
N = 66           # conditions (== pair categories P)
P = 66
CE = 24          # 2 * C_CAT
NCORES = 8
NL = 9           # conditions per core (66 -> 72 padded)
NPAD = NCORES * NL
BL = B // NCORES  # batch rows per core

KD = D // 128    # 8 k-tiles over D
KF = FI // 128   # 16 k-tiles over FEAT_IN

# weight dtype for the W_rep stream: bf16 (safe) or fp8 (half traffic)
WNAME = os.environ.get("CSN_WDT", "bf16")
WDT = FP8 if WNAME == "fp8" else BF16
WSCALE = float(os.environ.get("CSN_WSCALE", "128")) if WNAME == "fp8" else 1.0
# a2a group sizes (conditions per collective); first fires earliest
GROUPS = [int(x) for x in os.environ.get("CSN_GROUPS", "1,4,4").split(",")]
assert sum(GROUPS) == NL
# weight-pool prefetch depth (conditions)
WBUFS = int(os.environ.get("CSN_WBUFS", "8"))


def _split_multiwait_drains(nc):
    """This walrus build only accepts one sem wait per instruction; hoist
    extras onto NoOp carriers inserted just before the instruction (engines
    execute their stream in order, so wait-then-op is equivalent)."""
    fixno = 0
    for fnc in nc.m.functions:
        for bb in fnc.blocks:
            insts = bb.instructions
            i = 0
            while i < len(insts):
                inst = insts[i]
                si = inst.sync_info
                if si is not None and len(si.on_wait) > 1:
                    waits = list(si.on_wait)
                    si.on_wait = waits[-1:]
                    for w in waits[:-1]:
                        fixno += 1
                        carrier = mybir.InstNoOp(
                            name=f"I-waitfix-{fixno}",
                            engine=inst.engine,
                            ins=[],
                            outs=[],
                            sync_info=mybir.SyncInfo(on_wait=[w], on_update=[]),
                        )
                        insts.insert(i, carrier)
                        i += 1
                i += 1
    return fixno


def _build():
    nc = bass.Bass(
        "TRN2", target_bir_lowering=False, debug=False, num_devices=NCORES
    )
    ins = {
        "img_t": nc.dram_tensor("img_t", [128, KF, 128], BF16, kind="ExternalInput").ap(),
        "wemb_t": nc.dram_tensor("wemb_t", [128, KF, D], BF16, kind="ExternalInput").ap(),
        "b_emb": nc.dram_tensor("b_emb", [1, D], BF16, kind="ExternalInput").ap(),
        "w_rep_l": nc.dram_tensor("w_rep_l", [NL, D, D], WDT, kind="ExternalInput").ap(),
        "w1": nc.dram_tensor("w1", [CE, N], F32, kind="ExternalInput").ap(),
        "b1": nc.dram_tensor("b1", [1, N], F32, kind="ExternalInput").ap(),
        "w2": nc.dram_tensor("w2", [N, N], F32, kind="ExternalInput").ap(),
        "b2": nc.dram_tensor("b2", [1, N], F32, kind="ExternalInput").ap(),
        "cat_enc": nc.dram_tensor("cat_enc", [N, CE], F32, kind="ExternalInput").ap(),
        "perm_sel": nc.dram_tensor("perm_sel", [N, NPAD], F32, kind="ExternalInput").ap(),
    }
    out_cond = nc.dram_tensor(
        "out_cond", [P, BL * D], BF16, kind="ExternalOutput"
    ).ap()
    x_out = nc.dram_tensor("x_out", [B, D], F32, kind="ExternalOutput").ap()

    GS = list(GROUPS)
    N_OFF = [sum(GS[:g]) for g in range(len(GS))]
    R_OFF = [NCORES * o for o in N_OFF]
    # a2a payloads are bf16 packed into f32 words: collective time scales
    # with ELEMENT count (2048-elem CCE descriptors), not bytes
    sends = [
        nc.dram_tensor(f"a2a_send{g}", [NCORES, gs, BL, D // 2], F32)
        for g, gs in enumerate(GS)
    ]
    recvs = [
        nc.dram_tensor(f"a2a_recv{g}", [NCORES, gs, BL, D // 2], F32)
        for g, gs in enumerate(GS)
    ]

    with tile.TileContext(nc) as tc, tc.tile_pool(name="const", bufs=1) as cpool:
        id_sb = cpool.tile([128, 128], F32, name="id_sb")
        make_identity(nc, id_sb[:])

        # ---- tiny attention inputs (scalar ring, ~20 KB) -----------------
        ce_sb = cpool.tile([N, CE], F32, name="ce_sb")
        nc.scalar.dma_start(ce_sb[:], ins["cat_enc"][:])
        w1_sb = cpool.tile([CE, N], F32, name="w1_sb")
        nc.scalar.dma_start(w1_sb[:], ins["w1"][:])
        b1_sb = cpool.tile([1, N], F32, name="b1_sb")
        nc.scalar.dma_start(b1_sb[:], ins["b1"][:])
        w2_sb = cpool.tile([N, N], F32, name="w2_sb")
        nc.scalar.dma_start(w2_sb[:], ins["w2"][:])
        b2_sb = cpool.tile([1, N], F32, name="b2_sb")
        nc.scalar.dma_start(b2_sb[:], ins["b2"][:])
        psel_sb = cpool.tile([N, NPAD], F32, name="psel_sb")
        nc.scalar.dma_start(psel_sb[:], ins["perm_sel"][:])
        bemb_sb = cpool.tile([1, D], BF16, name="bemb_sb")
        nc.scalar.dma_start(bemb_sb[:], ins["b_emb"][:])

        # ---- pool scopes (LIFO): wpool outermost, startup pools inner ----
        from contextlib import ExitStack

        wstack = ExitStack()
        wpool = wstack.enter_context(tc.tile_pool(name="wpool", bufs=WBUFS))
        bstack = ExitStack()
        bpool = bstack.enter_context(tc.tile_pool(name="bpool", bufs=1))
        wembp = bstack.enter_context(tc.tile_pool(name="wembp", bufs=3))
        imgT_sb = bpool.tile([128, KF * 128], BF16, name="imgT_sb")
        nc.scalar.dma_start(
            imgT_sb[:].rearrange("p (t b) -> p t b", t=KF), ins["img_t"][:]
        )
        WEC = 4  # W_emb chunks of 4 k-tiles (1 MB each)
        wemb_ch = []
        for c in range(WEC):
            wc = wembp.tile([128, 4 * D], BF16, name="wemb", tag="wemb")
            nc.scalar.dma_start(
                wc[:].rearrange("p (t d) -> p t d", t=4),
                ins["wemb_t"][:, c * 4 : (c + 1) * 4, :],
            )
            wemb_ch.append(wc)

        # ---- W_rep weight stream: sync ring only, one 2 MB DMA / cond ----
        wtiles = []
        for n in range(NL):
            wt = wpool.tile([128, KD * D], WDT, name="wt", tag="wt")
            nc.sync.dma_start(
                wt[:].rearrange("p (k d) -> p k d", k=KD),
                ins["w_rep_l"][n].rearrange("(k p) d -> p k d", p=128),
            )
            wtiles.append(wt)

        # ---- ones row (f32 for att bias folds, bf16 for b_emb fold) ------
        onesA_sb = cpool.tile([1, 128], F32, name="onesA_sb")
        nc.vector.memset(onesA_sb[:], 1.0)
        ones_sb = cpool.tile([1, 128], BF16, name="ones_sb")
        nc.vector.tensor_copy(ones_sb[:], onesA_sb[:])

        # ---- phase A: attention matrix -> attT72 [NPAD, P] bf16 ----------
        attT72 = cpool.tile([NPAD, P], BF16, name="attT72")
        with tc.tile_pool(name="attp", bufs=1, space="PSUM") as attp:
            ceT_ps = attp.tile([CE, N], F32, name="ceT_ps")
            nc.tensor.transpose(ceT_ps[:], ce_sb[:], id_sb[:N, :N])
            ceT_sb = cpool.tile([CE, N], F32, name="ceT_sb")
            nc.vector.tensor_copy(ceT_sb[:], ceT_ps[:])

            h_ps = attp.tile([P, N], F32, name="h_ps")
            nc.tensor.matmul(h_ps[:], ceT_sb[:], w1_sb[:], start=True, stop=False)
            nc.tensor.matmul(h_ps[:], onesA_sb[:, :P], b1_sb[:], start=False, stop=True)
            h_sb = cpool.tile([P, N], F32, name="h_sb")
            nc.scalar.activation(h_sb[:], h_ps[:], mybir.ActivationFunctionType.Relu)

            hT_ps = attp.tile([N, P], F32, name="hT_ps")
            nc.tensor.transpose(hT_ps[:], h_sb[:], id_sb[:P, :P])
            hT_sb = cpool.tile([N, P], F32, name="hT_sb")
            nc.vector.tensor_copy(hT_sb[:], hT_ps[:])

            a_ps = attp.tile([P, N], F32, name="a_ps")
            nc.tensor.matmul(a_ps[:], hT_sb[:], w2_sb[:], start=True, stop=False)
            nc.tensor.matmul(a_ps[:], onesA_sb[:, :P], b2_sb[:], start=False, stop=True)
            att_sb = cpool.tile([P, N], F32, name="att_sb")
            nc.vector.tensor_copy(att_sb[:], a_ps[:])

            # row softmax
            rmax = cpool.tile([P, 1], F32, name="rmax")
            nc.vector.tensor_reduce(
                rmax[:], att_sb[:], axis=mybir.AxisListType.X, op=mybir.AluOpType.max
            )
            nc.vector.tensor_scalar_mul(rmax[:], rmax[:], -1.0)
            rsum = cpool.tile([P, 1], F32, name="rsum")
            nc.scalar.activation(
                att_sb[:],
                att_sb[:],
                mybir.ActivationFunctionType.Exp,
                bias=rmax[:],
                accum_out=rsum[:],
            )
            nc.vector.reciprocal(rsum[:], rsum[:])
            nc.vector.tensor_scalar_mul(att_sb[:], att_sb[:], rsum[:])

            attT_ps = attp.tile([N, P], F32, name="attT_ps")
            nc.tensor.transpose(attT_ps[:], att_sb[:], id_sb[:P, :P])
            attT_sb = cpool.tile([N, P], F32, name="attT_sb")
            nc.vector.tensor_copy(attT_sb[:], attT_ps[:])

            # permute att rows into R order (and fold 1/WSCALE, baked into
            # perm_sel on the host)
            attT72_ps = attp.tile([NPAD, P], F32, name="attT72_ps")
            nc.tensor.matmul(
                attT72_ps[:], psel_sb[:], attT_sb[:], start=True, stop=True
            )
            nc.vector.tensor_copy(attT72[:], attT72_ps[:])

        # ---- phase B: x = imgT.T @ W_emb + b_emb, then xT ---------------
        x_sb = cpool.tile([128, D], F32, name="x_sb")
        xT_sb = cpool.tile([128, D], BF16, name="xT_sb")  # 8 blocks [128d,128b]
        with (
            tc.tile_pool(name="bpsum", bufs=2, space="PSUM") as bpsum,
            tc.tile_pool(name="tpsum", bufs=2, space="PSUM") as tpsum,
        ):
            x_ps = [bpsum.tile([128, 512], F32, name=f"x_ps{h}") for h in range(2)]
            for k in range(KF):
                wc = wemb_ch[k // 4]
                kk = k % 4
                for h in range(2):
                    nc.tensor.matmul(
                        x_ps[h][:],
                        imgT_sb[:, k * 128 : (k + 1) * 128],
                        wc[:, kk * D + h * 512 : kk * D + (h + 1) * 512],
                        start=(k == 0),
                        stop=False,
                    )
            for h in range(2):
                nc.tensor.matmul(
                    x_ps[h][:],
                    ones_sb[:],
                    bemb_sb[:, h * 512 : (h + 1) * 512],
                    start=False,
                    stop=True,
                )
                nc.vector.tensor_copy(x_sb[:, h * 512 : (h + 1) * 512], x_ps[h][:])
            nc.scalar.dma_start(x_out[:], x_sb[:])
            for m in range(KD):
                tp = tpsum.tile([128, 128], F32, name="tp", tag="tp")
                nc.tensor.transpose(tp[:], x_sb[:, m * 128 : (m + 1) * 128], id_sb[:])
                nc.vector.tensor_copy(xT_sb[:, m * 128 : (m + 1) * 128], tp[:])
        bstack.close()

        # ---- phase C: grouped GEMM (k-outer per group) + a2a pipeline ----
        r_sb = cpool.tile([NPAD, BL * D], BF16, name="r_sb")
        with (
            tc.tile_pool(name="epool", bufs=3) as epool,
            tc.tile_pool(name="cpsum", bufs=1, space="PSUM") as cpsum,
        ):
            for g, gs in enumerate(GS):
                conds = list(range(N_OFF[g], N_OFF[g] + gs))
                e_ps = {
                    n: [
                        cpsum.tile(
                            [128, 512], F32, name="e_ps", tag=f"e{n % 4}_{h}"
                        )
                        for h in range(2)
                    ]
                    for n in conds
                }
                for k in range(KD):
                    for n in conds:
                        for h in range(2):
                            nc.tensor.matmul(
                                e_ps[n][h][:],
                                xT_sb[:, k * 128 : (k + 1) * 128],
                                wtiles[n][:, k * D + h * 512 : k * D + (h + 1) * 512],
                                start=(k == 0),
                                stop=(k == KD - 1),
                            )
                for n in conds:
                    e_sb = epool.tile([128, D], BF16, name="e_sb", tag="e_sb")
                    for h in range(2):
                        nc.vector.tensor_copy(
                            e_sb[:, h * 512 : (h + 1) * 512], e_ps[n][h][:]
                        )
                    nc.scalar.dma_start(
                        sends[g][:, n - N_OFF[g], :, :], e_sb[:].bitcast(F32)
                    )
                # collective doorbell for this group; recv loads issue
                # after every doorbell so the gpsimd FIFO never makes a
                # later collective wait on an earlier group's wire time
                nc.gpsimd.collective_compute(
                    "AllToAll",
                    mybir.AluOpType.bypass,
                    replica_groups=[list(range(NCORES))],
                    ins=[sends[g][:].opt()],
                    outs=[recvs[g][:].opt()],
                )
            for g, gs in enumerate(GS):
                rows = slice(R_OFF[g], R_OFF[g] + NCORES * gs)
                nc.gpsimd.dma_start(
                    r_sb[rows, :],
                    recvs[g][:].rearrange("a n b d -> (a n) (b d)").bitcast(BF16),
                )
        wstack.close()

        # ---- phase D: attention reduce + p-major bf16 output -------------
        with (
            tc.tile_pool(name="rpsum", bufs=1, space="PSUM") as rpsum,
            tc.tile_pool(name="spool", bufs=2) as spool,
        ):
            JC = 8  # j-chunks per output DMA (4 KB/partition)
            for jj in range(BL * 2 // JC):
                stg = spool.tile([P, JC * 512], BF16, name="stg", tag="stg")
                for j2 in range(JC):
                    j = jj * JC + j2
                    o_ps = rpsum.tile([P, 512], F32, name="o_ps", tag=f"o{j % 4}")
                    nc.tensor.matmul(
                        o_ps[:],
                        attT72[:],
                        r_sb[:, j * 512 : (j + 1) * 512],
                        start=True,
                        stop=True,
                    )
                    nc.vector.tensor_copy(
                        stg[:, j2 * 512 : (j2 + 1) * 512], o_ps[:]
                    )
                nc.scalar.dma_start(
                    out_cond[:, jj * JC * 512 : (jj + 1) * JC * 512], stg[:]
                )

    _split_multiwait_drains(nc)
    return nc


_NC_CACHE = {}
_LAST_IN_MAPS = None


def _get_nc():
    key = (WNAME, tuple(GROUPS))
    if key not in _NC_CACHE:
        _NC_CACHE[key] = _build()
    return _NC_CACHE[key]


def _host_att(W1, b1, W2, b2, cat_enc):
    h = np.maximum(cat_enc @ W1 + b1, 0.0)
    a = h @ W2 + b2
    a = a - a.max(axis=-1, keepdims=True)
    e = np.exp(a)
    return e / e.sum(axis=-1, keepdims=True)


def kernel(image, W_emb, b_emb, W_rep, b_rep, mask_table, W1, b1, W2, b2, cat_enc):
    import ml_dtypes

    f8 = ml_dtypes.float8_e4m3fn
    bf = ml_dtypes.bfloat16

    image = np.asarray(image, np.float32)
    W_emb = np.asarray(W_emb, np.float32)
    b_emb = np.asarray(b_emb, np.float32).reshape(1, D)
    W_rep = np.asarray(W_rep, np.float32)
    b_rep = np.asarray(b_rep, np.float32)
    mask_table = np.asarray(mask_table, np.float32)
    W1 = np.asarray(W1, np.float32)
    b1 = np.asarray(b1, np.float32).reshape(1, N)
    W2 = np.asarray(W2, np.float32)
    b2 = np.asarray(b2, np.float32).reshape(1, N)
    cat_enc = np.asarray(cat_enc, np.float32)

    # fold the mask (and fp8 scale) into the per-condition weights
    wm = W_rep * mask_table[:, None, :]          # [N, D, D] premasked
    if WSCALE != 1.0:
        wm = wm * WSCALE
    wrep_pad = np.zeros((NPAD, D, D), np.float32)
    wrep_pad[:N] = wm
    wrep_cast = wrep_pad.astype(f8 if WNAME == "fp8" else bf)

    # host-side layouts for the x GEMM
    img_t = np.ascontiguousarray(
        image.T.reshape(KF, 128, 128).transpose(1, 0, 2)
    ).astype(bf)                                  # [128, KF, 128]
    wemb_t = np.ascontiguousarray(
        W_emb.reshape(KF, 128, D).transpose(1, 0, 2)
    ).astype(bf)                                  # [128, KF, D]

    # R row r = R_OFF[g] + src*gs + gi holds condition 9*src + N_OFF[g] + gi
    GS = list(GROUPS)
    N_OFF = [sum(GS[:g]) for g in range(len(GS))]
    R_OFF = [NCORES * o for o in N_OFF]
    n_of_r = np.empty(NPAD, np.int64)
    for g in range(len(GS)):
        for src in range(NCORES):
            for gi in range(GS[g]):
                n_of_r[R_OFF[g] + src * GS[g] + gi] = NL * src + N_OFF[g] + gi
    psel = np.zeros((N, NPAD), np.float32)
    for r in range(NPAD):
        if n_of_r[r] < N:
            psel[n_of_r[r], r] = 1.0 / WSCALE

    nc = _get_nc()
    in_maps = []
    for i in range(NCORES):
        in_maps.append({
            "img_t": img_t,
            "wemb_t": wemb_t,
            "b_emb": b_emb.astype(bf),
            "w_rep_l": np.ascontiguousarray(wrep_cast[i * NL : (i + 1) * NL]),
            "w1": W1, "b1": b1, "w2": W2, "b2": b2,
            "cat_enc": cat_enc, "perm_sel": psel,
        })

    global _LAST_IN_MAPS
    _LAST_IN_MAPS = in_maps
    res = run_bass_kernel_spmd(nc, in_maps, list(range(NCORES)))

    out = np.empty((B, P + N, D), np.float32)
    for i in range(NCORES):
        oc = np.asarray(res.results[i]["out_cond"]).reshape(P, BL, D)
        out[i * BL : (i + 1) * BL, :P] = oc.transpose(1, 0, 2).astype(np.float32)
    x = np.asarray(res.results[0]["x_out"])
    out[:, P:] = x[:, None, :]

    # host correction for the (mask * b_rep) bias term (zero in practice)
    if np.any(b_rep):
        att = _host_att(W1, b1, W2, b2, cat_enc)
        corr = att @ (mask_table * b_rep)        # [P, D]
        out[:, :P] += corr[None]
    return out


# revision 13
# speedup vs baseline: 1.0617x; 1.0617x over previous
"""Trainium2 Bass kernel for ConditionalSimNet2 (moe_routing).

Computation (B=128, FEAT_IN=2048, D=1024, N=P=66 conditions):
    x          = image @ W_emb + b_emb                    [B, D]
    masked_rep = einsum('bd,nde->bne', x, W_rep) + b_rep  [B, N, D]
    embed      = mask_table * masked_rep                  [B, N, D]
    att        = softmax(relu(cat_enc@W1+b1)@W2 + b2)     [P, N]
    cond_feat  = einsum('pn,bnd->bpd', att, embed)        [B, P, D]
    out        = concat([cond_feat, broadcast(x)], 1)     [B, P+N, D]

Sharding: expert-parallel over the 66 conditions on 8 cores (9 each,
zero-padded to 72).  Every core computes x and att redundantly (cheap),
runs its 9 grouped GEMMs against its W_rep shard (the dominant HBM
traffic), exchanges embed slices with AllToAll so each core holds all
66 conditions for its 16-row batch shard, reduces with a single-K
matmul and writes its [66, 16*1024] output shard (p-major, bf16); the
host transposes/upcasts and broadcasts feature_x from core 0's x.

Key structural points vs the naive version:
  - mask_table (and the b_rep bias, via a host-side att-weighted
    correction) are folded into W_rep ON THE HOST, so the device never
    touches masks: no DVE broadcast-multiply, no bias matmuls in the
    hot loop.
  - image arrives pre-transposed/pre-cast (imgT bf16) and W_emb
    pre-cast bf16 k-major: the x GEMM is pure bf16 with no device-side
    image transposes.
  - grouped GEMM is k-outer over groups of conditions: the stationary
    operand (xT k-slice) is reused across the group, keeping the PE
    warm and LDWEIGHTS amortized.
  - DMA queues are specialized: sync ring = W_rep weight stream only
    (big FIFO 2 MB transfers at full HBM rate); scalar ring = startup
    loads + a2a sends + output writes; gpsimd = collectives + recv
    loads.  This keeps a blocked collective wait from ever stalling
    the weight stream or the next group's sends.
  - a2a payloads are bf16 (or whatever WDT the embed is cast to) and
    chunked [1,4,4] so the wire chain starts as early as possible.
  - W_rep can optionally ship in fp8e4m3 (scaled by WSCALE, de-scaled
    through the attention matrix) to halve the dominant HBM stream.
"""

import os
import sys

import numpy as np

try:
    import concourse.bass as bass
except ImportError:  # pragma: no cover - fallback when PYTHONPATH is not set
    sys.path.insert(0, "/opt/trn_rl_repo")
    import concourse.bass as bass

import concourse.mybir as mybir
import concourse.tile as tile
from concourse.bass_utils import run_bass_kernel_spmd
from concourse.masks import make_identity

F32 = mybir.dt.float32
BF16 = mybir.dt.bfloat16
FP8 = mybir.dt.float8e4

B = 128          # batch
FI = 2048        # backbone feature dim
D = 1024         # embed dim
N = 66           # conditions (== pair categories P)
P = 66
CE = 24          # 2 * C_CAT
NCORES = 8
NL = 9           # conditions per core (66 -> 72 padded)
NPAD = NCORES * NL
BL = B // NCORES  # batch rows per core

KD = D // 128    # 8 k-tiles over D
KF = FI // 128   # 16 k-tiles over FEAT_IN

# weight dtype for the W_rep stream: bf16 (safe) or fp8 (half traffic)
WNAME = os.environ.get("CSN_WDT", "bf16")
WDT = FP8 if WNAME == "fp8" else BF16
WSCALE = float(os.environ.get("CSN_WSCALE", "128")) if WNAME == "fp8" else 1.0
# a2a group sizes (conditions per collective); first fires earliest
GROUPS = [int(x) for x in os.environ.get("CSN_GROUPS", "1,4,4").split(",")]
assert sum(GROUPS) == NL
# weight-pool prefetch depth (conditions)
WBUFS = int(os.environ.get("CSN_WBUFS", "8"))


def _split_multiwait_drains(nc):
    """This walrus build only accepts one sem wait per instruction; hoist
    extras onto NoOp carriers inserted just before the instruction (engines
    execute their stream in order, so wait-then-op is equivalent)."""
    fixno = 0
    for fnc in nc.m.functions:
        for bb in fnc.blocks:
            insts = bb.instructions
            i = 0
            while i < len(insts):
                inst = insts[i]
                si = inst.sync_info
                if si is not None and len(si.on_wait) > 1:
                    waits = list(si.on_wait)
                    si.on_wait = waits[-1:]
                    for w in waits[:-1]:
                        fixno += 1
                        carrier = mybir.InstNoOp(
                            name=f"I-waitfix-{fixno}",
                            engine=inst.engine,
                            ins=[],
                            outs=[],
                            sync_info=mybir.SyncInfo(on_wait=[w], on_update=[]),
                        )
                        insts.insert(i, carrier)
                        i += 1
                i += 1
    return fixno


def _build():
    nc = bass.Bass(
        "TRN2", target_bir_lowering=False, debug=False, num_devices=NCORES
    )
    ins = {
        "img_t": nc.dram_tensor("img_t", [128, KF, 128], BF16, kind="ExternalInput").ap(),
        "wemb_t": nc.dram_tensor("wemb_t", [128, KF, D], BF16, kind="ExternalInput").ap(),
        "b_emb": nc.dram_tensor("b_emb", [1, D], BF16, kind="ExternalInput").ap(),
        "w_rep_l": nc.dram_tensor("w_rep_l", [NL, D, D], WDT, kind="ExternalInput").ap(),
        "w1": nc.dram_tensor("w1", [CE, N], F32, kind="ExternalInput").ap(),
        "b1": nc.dram_tensor("b1", [1, N], F32, kind="ExternalInput").ap(),
        "w2": nc.dram_tensor("w2", [N, N], F32, kind="ExternalInput").ap(),
        "b2": nc.dram_tensor("b2", [1, N], F32, kind="ExternalInput").ap(),
        "cat_enc": nc.dram_tensor("cat_enc", [N, CE], F32, kind="ExternalInput").ap(),
        "perm_sel": nc.dram_tensor("perm_sel", [N, NPAD], F32, kind="ExternalInput").ap(),
    }
    out_cond = nc.dram_tensor(
        "out_cond", [P, BL * D], BF16, kind="ExternalOutput"
    ).ap()
    x_out = nc.dram_tensor("x_out", [B, D], F32, kind="ExternalOutput").ap()

    GS = list(GROUPS)
    N_OFF = [sum(GS[:g]) for g in range(len(GS))]
    R_OFF = [NCORES * o for o in N_OFF]
    # a2a payloads are bf16 packed into f32 words: collective time scales
    # with ELEMENT count (2048-elem CCE descriptors), not bytes
    sends = [
        nc.dram_tensor(f"a2a_send{g}", [NCORES, gs, BL, D // 2], F32)
        for g, gs in enumerate(GS)
    ]
    recvs = [
        nc.dram_tensor(f"a2a_recv{g}", [NCORES, gs, BL, D // 2], F32)
        for g, gs in enumerate(GS)
    ]

    with tile.TileContext(nc) as tc, tc.tile_pool(name="const", bufs=1) as cpool:
        id_sb = cpool.tile([128, 128], F32, name="id_sb")
        make_identity(nc, id_sb[:])

        # ---- pool scopes (LIFO): wpool outermost, startup pools inner ----
        from contextlib import ExitStack

        wstack = ExitStack()
        wpool = wstack.enter_context(tc.tile_pool(name="wpool", bufs=WBUFS))
        bstack = ExitStack()
        bpool = bstack.enter_context(tc.tile_pool(name="bpool", bufs=1))
        wembp = bstack.enter_context(tc.tile_pool(name="wembp", bufs=3))
        imgT_sb = bpool.tile([128, KF * 128], BF16, name="imgT_sb")
        nc.scalar.dma_start(
            imgT_sb[:].rearrange("p (t b) -> p t b", t=KF), ins["img_t"][:]
        )
        WEC = 4  # W_emb chunks of 4 k-tiles (1 MB each)
        wemb_ch = []
        for c in range(WEC):
            wc = wembp.tile([128, 4 * D], BF16, name="wemb", tag="wemb")
            eng = nc.sync if c < 2 else nc.scalar
            eng.dma_start(
                wc[:].rearrange("p (t d) -> p t d", t=4),
                ins["wemb_t"][:, c * 4 : (c + 1) * 4, :],
            )
            wemb_ch.append(wc)

        # tiny attention inputs (behind the startup loads; att is not urgent)
        ce_sb = cpool.tile([N, CE], F32, name="ce_sb")
        nc.scalar.dma_start(ce_sb[:], ins["cat_enc"][:])
        w1_sb = cpool.tile([CE, N], F32, name="w1_sb")
        nc.scalar.dma_start(w1_sb[:], ins["w1"][:])
        b1_sb = cpool.tile([1, N], F32, name="b1_sb")
        nc.scalar.dma_start(b1_sb[:], ins["b1"][:])
        w2_sb = cpool.tile([N, N], F32, name="w2_sb")
        nc.scalar.dma_start(w2_sb[:], ins["w2"][:])
        b2_sb = cpool.tile([1, N], F32, name="b2_sb")
        nc.scalar.dma_start(b2_sb[:], ins["b2"][:])
        psel_sb = cpool.tile([N, NPAD], F32, name="psel_sb")
        nc.scalar.dma_start(psel_sb[:], ins["perm_sel"][:])
        bemb_sb = cpool.tile([1, D], BF16, name="bemb_sb")
        nc.scalar.dma_start(bemb_sb[:], ins["b_emb"][:])

        # ---- W_rep weight stream: sync ring only, one 2 MB DMA / cond ----
        wtiles = []
        for n in range(NL):
            wt = wpool.tile([128, KD * D], WDT, name="wt", tag="wt")
            nc.sync.dma_start(
                wt[:].rearrange("p (k d) -> p k d", k=KD),
                ins["w_rep_l"][n].rearrange("(k p) d -> p k d", p=128),
            )
            wtiles.append(wt)

        # ---- ones row (f32 for att bias folds, bf16 for b_emb fold) ------
        onesA_sb = cpool.tile([1, 128], F32, name="onesA_sb")
        nc.vector.memset(onesA_sb[:], 1.0)
        ones_sb = cpool.tile([1, 128], BF16, name="ones_sb")
        nc.vector.tensor_copy(ones_sb[:], onesA_sb[:])

        # ---- phase A: attention matrix -> attT72 [NPAD, P] bf16 ----------
        attT72 = cpool.tile([NPAD, P], BF16, name="attT72")
        with tc.tile_pool(name="attp", bufs=1, space="PSUM") as attp:
            ceT_ps = attp.tile([CE, N], F32, name="ceT_ps")
            nc.tensor.transpose(ceT_ps[:], ce_sb[:], id_sb[:N, :N])
            ceT_sb = cpool.tile([CE, N], F32, name="ceT_sb")
            nc.vector.tensor_copy(ceT_sb[:], ceT_ps[:])

            h_ps = attp.tile([P, N], F32, name="h_ps")
            nc.tensor.matmul(h_ps[:], ceT_sb[:], w1_sb[:], start=True, stop=False)
            nc.tensor.matmul(h_ps[:], onesA_sb[:, :P], b1_sb[:], start=False, stop=True)
            h_sb = cpool.tile([P, N], F32, name="h_sb")
            nc.scalar.activation(h_sb[:], h_ps[:], mybir.ActivationFunctionType.Relu)

            hT_ps = attp.tile([N, P], F32, name="hT_ps")
            nc.tensor.transpose(hT_ps[:], h_sb[:], id_sb[:P, :P])
            hT_sb = cpool.tile([N, P], F32, name="hT_sb")
            nc.vector.tensor_copy(hT_sb[:], hT_ps[:])

            a_ps = attp.tile([P, N], F32, name="a_ps")
            nc.tensor.matmul(a_ps[:], hT_sb[:], w2_sb[:], start=True, stop=False)
            nc.tensor.matmul(a_ps[:], onesA_sb[:, :P], b2_sb[:], start=False, stop=True)
            att_sb = cpool.tile([P, N], F32, name="att_sb")
            nc.vector.tensor_copy(att_sb[:], a_ps[:])

            # row softmax
            rmax = cpool.tile([P, 1], F32, name="rmax")
            nc.vector.tensor_reduce(
                rmax[:], att_sb[:], axis=mybir.AxisListType.X, op=mybir.AluOpType.max
            )
            nc.vector.tensor_scalar_mul(rmax[:], rmax[:], -1.0)
            rsum = cpool.tile([P, 1], F32, name="rsum")
            nc.scalar.activation(
                att_sb[:],
                att_sb[:],
                mybir.ActivationFunctionType.Exp,
                bias=rmax[:],
                accum_out=rsum[:],
            )
            nc.vector.reciprocal(rsum[:], rsum[:])
            nc.vector.tensor_scalar_mul(att_sb[:], att_sb[:], rsum[:])

            attT_ps = attp.tile([N, P], F32, name="attT_ps")
            nc.tensor.transpose(attT_ps[:], att_sb[:], id_sb[:P, :P])
            attT_sb = cpool.tile([N, P], F32, name="attT_sb")
            nc.vector.tensor_copy(attT_sb[:], attT_ps[:])

            # permute att rows into R order (and fold 1/WSCALE, baked into
            # perm_sel on the host)
            attT72_ps = attp.tile([NPAD, P], F32, name="attT72_ps")
            nc.tensor.matmul(
                attT72_ps[:], psel_sb[:], attT_sb[:], start=True, stop=True
            )
            nc.vector.tensor_copy(attT72[:], attT72_ps[:])

        # ---- phase B: x = imgT.T @ W_emb + b_emb, then xT ---------------
        x_sb = cpool.tile([128, D], F32, name="x_sb")
        xT_sb = cpool.tile([128, D], BF16, name="xT_sb")  # 8 blocks [128d,128b]
        with (
            tc.tile_pool(name="bpsum", bufs=2, space="PSUM") as bpsum,
            tc.tile_pool(name="tpsum", bufs=2, space="PSUM") as tpsum,
        ):
            x_ps = [bpsum.tile([128, 512], F32, name=f"x_ps{h}") for h in range(2)]
            for k in range(KF):
                wc = wemb_ch[k // 4]
                kk = k % 4
                for h in range(2):
                    nc.tensor.matmul(
                        x_ps[h][:],
                        imgT_sb[:, k * 128 : (k + 1) * 128],
                        wc[:, kk * D + h * 512 : kk * D + (h + 1) * 512],
                        start=(k == 0),
                        stop=False,
                    )
            for h in range(2):
                nc.tensor.matmul(
                    x_ps[h][:],
                    ones_sb[:],
                    bemb_sb[:, h * 512 : (h + 1) * 512],
                    start=False,
                    stop=True,
                )
                nc.vector.tensor_copy(x_sb[:, h * 512 : (h + 1) * 512], x_ps[h][:])
            nc.scalar.dma_start(x_out[:], x_sb[:])
            for m in range(KD):
                tp = tpsum.tile([128, 128], F32, name="tp", tag="tp")
                nc.tensor.transpose(tp[:], x_sb[:, m * 128 : (m + 1) * 128], id_sb[:])
                nc.vector.tensor_copy(xT_sb[:, m * 128 : (m + 1) * 128], tp[:])
        bstack.close()

        # ---- phase C: grouped GEMM (k-outer per group) + a2a pipeline ----
        r_sb = cpool.tile([NPAD, BL * D], BF16, name="r_sb")
        with (
            tc.tile_pool(name="epool", bufs=3) as epool,
            tc.tile_pool(name="cpsum", bufs=1, space="PSUM") as cpsum,
        ):
            for g, gs in enumerate(GS):
                conds = list(range(N_OFF[g], N_OFF[g] + gs))
                e_ps = {
                    n: [
                        cpsum.tile(
                            [128, 512], F32, name="e_ps", tag=f"e{n % 4}_{h}"
                        )
                        for h in range(2)
                    ]
                    for n in conds
                }
                for k in range(KD):
                    for n in conds:
                        for h in range(2):
                            nc.tensor.matmul(
                                e_ps[n][h][:],
                                xT_sb[:, k * 128 : (k + 1) * 128],
                                wtiles[n][:, k * D + h * 512 : k * D + (h + 1) * 512],
                                start=(k == 0),
                                stop=(k == KD - 1),
                            )
                for n in conds:
                    e_sb = epool.tile([128, D], BF16, name="e_sb", tag="e_sb")
                    for h in range(2):
                        nc.vector.tensor_copy(
                            e_sb[:, h * 512 : (h + 1) * 512], e_ps[n][h][:]
                        )
                    nc.scalar.dma_start(
                        sends[g][:, n - N_OFF[g], :, :], e_sb[:].bitcast(F32)
                    )
                # collective doorbell for this group; recv loads issue
                # after every doorbell so the gpsimd FIFO never makes a
                # later collective wait on an earlier group's wire time
                nc.gpsimd.collective_compute(
                    "AllToAll",
                    mybir.AluOpType.bypass,
                    replica_groups=[list(range(NCORES))],
                    ins=[sends[g][:].opt()],
                    outs=[recvs[g][:].opt()],
                )
            for g, gs in enumerate(GS):
                rows = slice(R_OFF[g], R_OFF[g] + NCORES * gs)
                nc.gpsimd.dma_start(
                    r_sb[rows, :],
                    recvs[g][:].rearrange("a n b d -> (a n) (b d)").bitcast(BF16),
                )
        wstack.close()

        # ---- phase D: attention reduce + p-major bf16 output -------------
        with (
            tc.tile_pool(name="rpsum", bufs=1, space="PSUM") as rpsum,
            tc.tile_pool(name="spool", bufs=2) as spool,
        ):
            JC = 8  # j-chunks per output DMA (4 KB/partition)
            for jj in range(BL * 2 // JC):
                stg = spool.tile([P, JC * 512], BF16, name="stg", tag="stg")
                for j2 in range(JC):
                    j = jj * JC + j2
                    o_ps = rpsum.tile([P, 512], F32, name="o_ps", tag=f"o{j % 4}")
                    nc.tensor.matmul(
                        o_ps[:],
                        attT72[:],
                        r_sb[:, j * 512 : (j + 1) * 512],
                        start=True,
                        stop=True,
                    )
                    nc.vector.tensor_copy(
                        stg[:, j2 * 512 : (j2 + 1) * 512], o_ps[:]
                    )
                nc.scalar.dma_start(
                    out_cond[:, jj * JC * 512 : (jj + 1) * JC * 512], stg[:]
                )

    _split_multiwait_drains(nc)
    return nc


_NC_CACHE = {}
_LAST_IN_MAPS = None


def _get_nc():
    key = (WNAME, tuple(GROUPS))
    if key not in _NC_CACHE:
        _NC_CACHE[key] = _build()
    return _NC_CACHE[key]


def _host_att(W1, b1, W2, b2, cat_enc):
    h = np.maximum(cat_enc @ W1 + b1, 0.0)
    a = h @ W2 + b2
    a = a - a.max(axis=-1, keepdims=True)
    e = np.exp(a)
    return e / e.sum(axis=-1, keepdims=True)


def kernel(image, W_emb, b_emb, W_rep, b_rep, mask_table, W1, b1, W2, b2, cat_enc):
    import ml_dtypes

    f8 = ml_dtypes.float8_e4m3fn
    bf = ml_dtypes.bfloat16

    image = np.asarray(image, np.float32)
    W_emb = np.asarray(W_emb, np.float32)
    b_emb = np.asarray(b_emb, np.float32).reshape(1, D)
    W_rep = np.asarray(W_rep, np.float32)
    b_rep = np.asarray(b_rep, np.float32)
    mask_table = np.asarray(mask_table, np.float32)
    W1 = np.asarray(W1, np.float32)
    b1 = np.asarray(b1, np.float32).reshape(1, N)
    W2 = np.asarray(W2, np.float32)
    b2 = np.asarray(b2, np.float32).reshape(1, N)
    cat_enc = np.asarray(cat_enc, np.float32)

    # fold the mask (and fp8 scale) into the per-condition weights
    wm = W_rep * mask_table[:, None, :]          # [N, D, D] premasked
    if WSCALE != 1.0:
        wm = wm * WSCALE
    wrep_pad = np.zeros((NPAD, D, D), np.float32)
    wrep_pad[:N] = wm
    wrep_cast = wrep_pad.astype(f8 if WNAME == "fp8" else bf)

    # host-side layouts for the x GEMM
    img_t = np.ascontiguousarray(
        image.T.reshape(KF, 128, 128).transpose(1, 0, 2)
    ).astype(bf)                                  # [128, KF, 128]
    wemb_t = np.ascontiguousarray(
        W_emb.reshape(KF, 128, D).transpose(1, 0, 2)
    ).astype(bf)                                  # [128, KF, D]

    # R row r = R_OFF[g] + src*gs + gi holds condition 9*src + N_OFF[g] + gi
    GS = list(GROUPS)
    N_OFF = [sum(GS[:g]) for g in range(len(GS))]
    R_OFF = [NCORES * o for o in N_OFF]
    n_of_r = np.empty(NPAD, np.int64)
    for g in range(len(GS)):
        for src in range(NCORES):
            for gi in range(GS[g]):
                n_of_r[R_OFF[g] + src * GS[g] + gi] = NL * src + N_OFF[g] + gi
    psel = np.zeros((N, NPAD), np.float32)
    for r in range(NPAD):
        if n_of_r[r] < N:
            psel[n_of_r[r], r] = 1.0 / WSCALE

    nc = _get_nc()
    in_maps = []
    for i in range(NCORES):
        in_maps.append({
            "img_t": img_t,
            "wemb_t": wemb_t,
            "b_emb": b_emb.astype(bf),
            "w_rep_l": np.ascontiguousarray(wrep_cast[i * NL : (i + 1) * NL]),
            "w1": W1, "b1": b1, "w2": W2, "b2": b2,
            "cat_enc": cat_enc, "perm_sel": psel,
        })

    global _LAST_IN_MAPS
    _LAST_IN_MAPS = in_maps
    res = run_bass_kernel_spmd(nc, in_maps, list(range(NCORES)))

    out = np.empty((B, P + N, D), np.float32)
    for i in range(NCORES):
        oc = np.asarray(res.results[i]["out_cond"]).reshape(P, BL, D)
        out[i * BL : (i + 1) * BL, :P] = oc.transpose(1, 0, 2).astype(np.float32)
    x = np.asarray(res.results[0]["x_out"])
    out[:, P:] = x[:, None, :]

    # host correction for the (mask * b_rep) bias term (zero in practice)
    if np.any(b_rep):
        att = _host_att(W1, b1, W2, b2, cat_enc)
        corr = att @ (mask_table * b_rep)        # [P, D]
        out[:, :P] += corr[None]
    return out


# revision 14
# speedup vs baseline: 1.0932x; 1.0297x over previous
"""Trainium2 Bass kernel for ConditionalSimNet2 (moe_routing).

Computation (B=128, FEAT_IN=2048, D=1024, N=P=66 conditions):
    x          = image @ W_emb + b_emb                    [B, D]
    masked_rep = einsum('bd,nde->bne', x, W_rep) + b_rep  [B, N, D]
    embed      = mask_table * masked_rep                  [B, N, D]
    att        = softmax(relu(cat_enc@W1+b1)@W2 + b2)     [P, N]
    cond_feat  = einsum('pn,bnd->bpd', att, embed)        [B, P, D]
    out        = concat([cond_feat, broadcast(x)], 1)     [B, P+N, D]

Sharding: expert-parallel over the 66 conditions on 8 cores (9 each,
zero-padded to 72).  Every core computes x and att redundantly (cheap),
runs its 9 grouped GEMMs against its W_rep shard (the dominant HBM
traffic), exchanges embed slices with AllToAll so each core holds all
66 conditions for its 16-row batch shard, reduces with a single-K
matmul and writes its [66, 16*1024] output shard (p-major, bf16); the
host transposes/upcasts and broadcasts feature_x from core 0's x.

Key structural points vs the naive version:
  - mask_table (and the b_rep bias, via a host-side att-weighted
    correction) are folded into W_rep ON THE HOST, so the device never
    touches masks: no DVE broadcast-multiply, no bias matmuls in the
    hot loop.
  - image arrives pre-transposed/pre-cast (imgT bf16) and W_emb
    pre-cast bf16 k-major: the x GEMM is pure bf16 with no device-side
    image transposes.
  - grouped GEMM is k-outer over groups of conditions: the stationary
    operand (xT k-slice) is reused across the group, keeping the PE
    warm and LDWEIGHTS amortized.
  - DMA queues are specialized: sync ring = W_rep weight stream only
    (big FIFO 2 MB transfers at full HBM rate); scalar ring = startup
    loads + a2a sends + output writes; gpsimd = collectives + recv
    loads.  This keeps a blocked collective wait from ever stalling
    the weight stream or the next group's sends.
  - a2a payloads are bf16 (or whatever WDT the embed is cast to) and
    chunked [1,4,4] so the wire chain starts as early as possible.
  - W_rep can optionally ship in fp8e4m3 (scaled by WSCALE, de-scaled
    through the attention matrix) to halve the dominant HBM stream.
"""

import os
import sys

import numpy as np

try:
    import concourse.bass as bass
except ImportError:  # pragma: no cover - fallback when PYTHONPATH is not set
    sys.path.insert(0, "/opt/trn_rl_repo")
    import concourse.bass as bass

import concourse.mybir as mybir
import concourse.tile as tile
from concourse.bass_utils import run_bass_kernel_spmd
from concourse.masks import make_identity

F32 = mybir.dt.float32
BF16 = mybir.dt.bfloat16
FP8 = mybir.dt.float8e4

B = 128          # batch
FI = 2048        # backbone feature dim
D = 1024         # embed dim
N = 66           # conditions (== pair categories P)
P = 66
CE = 24          # 2 * C_CAT
NCORES = 8
NL = 9           # conditions per core (66 -> 72 padded)
NPAD = NCORES * NL
BL = B // NCORES  # batch rows per core

KD = D // 128    # 8 k-tiles over D
KF = FI // 128   # 16 k-tiles over FEAT_IN

# weight dtype for the W_rep stream: bf16 (safe) or fp8 (half traffic)
WNAME = os.environ.get("CSN_WDT", "bf16")
WDT = FP8 if WNAME == "fp8" else BF16
WSCALE = float(os.environ.get("CSN_WSCALE", "128")) if WNAME == "fp8" else 1.0
# a2a group sizes (conditions per collective); first fires earliest
GROUPS = [int(x) for x in os.environ.get("CSN_GROUPS", "1,4,4").split(",")]
assert sum(GROUPS) == NL
# weight-pool prefetch depth (conditions)
WBUFS = int(os.environ.get("CSN_WBUFS", "8"))


def _split_multiwait_drains(nc):
    """This walrus build only accepts one sem wait per instruction; hoist
    extras onto NoOp carriers inserted just before the instruction (engines
    execute their stream in order, so wait-then-op is equivalent)."""
    fixno = 0
    for fnc in nc.m.functions:
        for bb in fnc.blocks:
            insts = bb.instructions
            i = 0
            while i < len(insts):
                inst = insts[i]
                si = inst.sync_info
                if si is not None and len(si.on_wait) > 1:
                    waits = list(si.on_wait)
                    si.on_wait = waits[-1:]
                    for w in waits[:-1]:
                        fixno += 1
                        carrier = mybir.InstNoOp(
                            name=f"I-waitfix-{fixno}",
                            engine=inst.engine,
                            ins=[],
                            outs=[],
                            sync_info=mybir.SyncInfo(on_wait=[w], on_update=[]),
                        )
                        insts.insert(i, carrier)
                        i += 1
                i += 1
    return fixno


def _build():
    nc = bass.Bass(
        "TRN2", target_bir_lowering=False, debug=False, num_devices=NCORES
    )
    ins = {
        "img_t": nc.dram_tensor("img_t", [128, KF, 128], BF16, kind="ExternalInput").ap(),
        "wemb_t": nc.dram_tensor("wemb_t", [128, KF, D], BF16, kind="ExternalInput").ap(),
        "b_emb": nc.dram_tensor("b_emb", [1, D], BF16, kind="ExternalInput").ap(),
        "w_rep_l": nc.dram_tensor("w_rep_l", [NL, D, D], WDT, kind="ExternalInput").ap(),
        "w1": nc.dram_tensor("w1", [CE, N], F32, kind="ExternalInput").ap(),
        "b1": nc.dram_tensor("b1", [1, N], F32, kind="ExternalInput").ap(),
        "w2": nc.dram_tensor("w2", [N, N], F32, kind="ExternalInput").ap(),
        "b2": nc.dram_tensor("b2", [1, N], F32, kind="ExternalInput").ap(),
        "cat_enc": nc.dram_tensor("cat_enc", [N, CE], F32, kind="ExternalInput").ap(),
        "perm_sel": nc.dram_tensor("perm_sel", [N, NPAD], F32, kind="ExternalInput").ap(),
    }
    out_cond = nc.dram_tensor(
        "out_cond", [P, BL * D], BF16, kind="ExternalOutput"
    ).ap()
    x_out = nc.dram_tensor("x_out", [B, D], F32, kind="ExternalOutput").ap()

    GS = list(GROUPS)
    N_OFF = [sum(GS[:g]) for g in range(len(GS))]
    R_OFF = [NCORES * o for o in N_OFF]
    # a2a payloads are bf16 packed into f32 words: collective time scales
    # with ELEMENT count (2048-elem CCE descriptors), not bytes
    sends = [
        nc.dram_tensor(f"a2a_send{g}", [NCORES, gs, BL, D // 2], F32)
        for g, gs in enumerate(GS)
    ]
    recvs = [
        nc.dram_tensor(f"a2a_recv{g}", [NCORES, gs, BL, D // 2], F32)
        for g, gs in enumerate(GS)
    ]

    with tile.TileContext(nc) as tc, tc.tile_pool(name="const", bufs=1) as cpool:
        id_sb = cpool.tile([128, 128], F32, name="id_sb")
        make_identity(nc, id_sb[:])

        # ---- pool scopes (LIFO): wpool outermost, startup pools inner ----
        from contextlib import ExitStack

        wstack = ExitStack()
        wpool = wstack.enter_context(tc.tile_pool(name="wpool", bufs=WBUFS))
        bstack = ExitStack()
        bpool = bstack.enter_context(tc.tile_pool(name="bpool", bufs=1))
        wembp = bstack.enter_context(tc.tile_pool(name="wembp", bufs=3))
        imgT_sb = bpool.tile([128, KF * 128], BF16, name="imgT_sb")
        nc.scalar.dma_start(
            imgT_sb[:].rearrange("p (t b) -> p t b", t=KF), ins["img_t"][:]
        )
        WEC = 4  # W_emb chunks of 4 k-tiles (1 MB each)
        wemb_ch = []
        for c in range(WEC):
            wc = wembp.tile([128, 4 * D], BF16, name="wemb", tag="wemb")
            eng = nc.sync if c < 2 else nc.scalar
            eng.dma_start(
                wc[:].rearrange("p (t d) -> p t d", t=4),
                ins["wemb_t"][:, c * 4 : (c + 1) * 4, :],
            )
            wemb_ch.append(wc)

        # tiny attention inputs (behind the startup loads; att is not urgent)
        ce_sb = cpool.tile([N, CE], F32, name="ce_sb")
        nc.scalar.dma_start(ce_sb[:], ins["cat_enc"][:])
        w1_sb = cpool.tile([CE, N], F32, name="w1_sb")
        nc.scalar.dma_start(w1_sb[:], ins["w1"][:])
        b1_sb = cpool.tile([1, N], F32, name="b1_sb")
        nc.scalar.dma_start(b1_sb[:], ins["b1"][:])
        w2_sb = cpool.tile([N, N], F32, name="w2_sb")
        nc.scalar.dma_start(w2_sb[:], ins["w2"][:])
        b2_sb = cpool.tile([1, N], F32, name="b2_sb")
        nc.scalar.dma_start(b2_sb[:], ins["b2"][:])
        psel_sb = cpool.tile([N, NPAD], F32, name="psel_sb")
        nc.scalar.dma_start(psel_sb[:], ins["perm_sel"][:])
        bemb_sb = cpool.tile([1, D], BF16, name="bemb_sb")
        nc.scalar.dma_start(bemb_sb[:], ins["b_emb"][:])

        # ---- W_rep weight stream: sync ring only, one 2 MB DMA / cond ----
        wtiles = []
        for n in range(NL):
            wt = wpool.tile([128, KD * D], WDT, name="wt", tag="wt")
            nc.sync.dma_start(
                wt[:].rearrange("p (k d) -> p k d", k=KD),
                ins["w_rep_l"][n].rearrange("(k p) d -> p k d", p=128),
            )
            wtiles.append(wt)

        # ---- ones row (f32 for att bias folds, bf16 for b_emb fold) ------
        onesA_sb = cpool.tile([1, 128], F32, name="onesA_sb")
        nc.vector.memset(onesA_sb[:], 1.0)
        ones_sb = cpool.tile([1, 128], BF16, name="ones_sb")
        nc.vector.tensor_copy(ones_sb[:], onesA_sb[:])

        # ---- phase A: attention matrix -> attT72 [NPAD, P] bf16 ----------
        attT72 = cpool.tile([NPAD, P], BF16, name="attT72")
        with tc.tile_pool(name="attp", bufs=1, space="PSUM") as attp:
            ceT_ps = attp.tile([CE, N], F32, name="ceT_ps")
            nc.tensor.transpose(ceT_ps[:], ce_sb[:], id_sb[:N, :N])
            ceT_sb = cpool.tile([CE, N], F32, name="ceT_sb")
            nc.vector.tensor_copy(ceT_sb[:], ceT_ps[:])

            h_ps = attp.tile([P, N], F32, name="h_ps")
            nc.tensor.matmul(h_ps[:], ceT_sb[:], w1_sb[:], start=True, stop=False)
            nc.tensor.matmul(h_ps[:], onesA_sb[:, :P], b1_sb[:], start=False, stop=True)
            h_sb = cpool.tile([P, N], F32, name="h_sb")
            nc.scalar.activation(h_sb[:], h_ps[:], mybir.ActivationFunctionType.Relu)

            hT_ps = attp.tile([N, P], F32, name="hT_ps")
            nc.tensor.transpose(hT_ps[:], h_sb[:], id_sb[:P, :P])
            hT_sb = cpool.tile([N, P], F32, name="hT_sb")
            nc.vector.tensor_copy(hT_sb[:], hT_ps[:])

            a_ps = attp.tile([P, N], F32, name="a_ps")
            nc.tensor.matmul(a_ps[:], hT_sb[:], w2_sb[:], start=True, stop=False)
            nc.tensor.matmul(a_ps[:], onesA_sb[:, :P], b2_sb[:], start=False, stop=True)
            att_sb = cpool.tile([P, N], F32, name="att_sb")
            nc.vector.tensor_copy(att_sb[:], a_ps[:])

            # row softmax
            rmax = cpool.tile([P, 1], F32, name="rmax")
            nc.vector.tensor_reduce(
                rmax[:], att_sb[:], axis=mybir.AxisListType.X, op=mybir.AluOpType.max
            )
            nc.vector.tensor_scalar_mul(rmax[:], rmax[:], -1.0)
            rsum = cpool.tile([P, 1], F32, name="rsum")
            nc.scalar.activation(
                att_sb[:],
                att_sb[:],
                mybir.ActivationFunctionType.Exp,
                bias=rmax[:],
                accum_out=rsum[:],
            )
            nc.vector.reciprocal(rsum[:], rsum[:])
            nc.vector.tensor_scalar_mul(att_sb[:], att_sb[:], rsum[:])

            attT_ps = attp.tile([N, P], F32, name="attT_ps")
            nc.tensor.transpose(attT_ps[:], att_sb[:], id_sb[:P, :P])
            attT_sb = cpool.tile([N, P], F32, name="attT_sb")
            nc.vector.tensor_copy(attT_sb[:], attT_ps[:])

            # permute att rows into R order (and fold 1/WSCALE, baked into
            # perm_sel on the host)
            attT72_ps = attp.tile([NPAD, P], F32, name="attT72_ps")
            nc.tensor.matmul(
                attT72_ps[:], psel_sb[:], attT_sb[:], start=True, stop=True
            )
            nc.vector.tensor_copy(attT72[:], attT72_ps[:])

        # ---- phase B: x = imgT.T @ W_emb + b_emb, then xT ---------------
        x_sb = cpool.tile([128, D], F32, name="x_sb")
        xT_sb = cpool.tile([128, D], BF16, name="xT_sb")  # 8 blocks [128d,128b]
        with (
            tc.tile_pool(name="bpsum", bufs=2, space="PSUM") as bpsum,
            tc.tile_pool(name="tpsum", bufs=2, space="PSUM") as tpsum,
        ):
            x_ps = [bpsum.tile([128, 512], F32, name=f"x_ps{h}") for h in range(2)]
            for k in range(KF):
                wc = wemb_ch[k // 4]
                kk = k % 4
                for h in range(2):
                    nc.tensor.matmul(
                        x_ps[h][:],
                        imgT_sb[:, k * 128 : (k + 1) * 128],
                        wc[:, kk * D + h * 512 : kk * D + (h + 1) * 512],
                        start=(k == 0),
                        stop=False,
                    )
            for h in range(2):
                nc.tensor.matmul(
                    x_ps[h][:],
                    ones_sb[:],
                    bemb_sb[:, h * 512 : (h + 1) * 512],
                    start=False,
                    stop=True,
                )
                nc.vector.tensor_copy(x_sb[:, h * 512 : (h + 1) * 512], x_ps[h][:])
            nc.scalar.dma_start(x_out[:], x_sb[:])
            for m in range(KD):
                tp = tpsum.tile([128, 128], F32, name="tp", tag="tp")
                nc.tensor.transpose(tp[:], x_sb[:, m * 128 : (m + 1) * 128], id_sb[:])
                nc.vector.tensor_copy(xT_sb[:, m * 128 : (m + 1) * 128], tp[:])
        bstack.close()

        # ---- phase C: grouped GEMM (k-outer per group) + a2a pipeline ----
        r_sb = cpool.tile([NPAD, BL * D], BF16, name="r_sb")
        with (
            tc.tile_pool(name="epool", bufs=3) as epool,
            tc.tile_pool(name="cpsum", bufs=1, space="PSUM") as cpsum,
        ):
            for g, gs in enumerate(GS):
                conds = list(range(N_OFF[g], N_OFF[g] + gs))
                e_ps = {
                    n: [
                        cpsum.tile(
                            [128, 512], F32, name="e_ps", tag=f"e{n % 4}_{h}"
                        )
                        for h in range(2)
                    ]
                    for n in conds
                }
                for k in range(KD):
                    for n in conds:
                        for h in range(2):
                            nc.tensor.matmul(
                                e_ps[n][h][:],
                                xT_sb[:, k * 128 : (k + 1) * 128],
                                wtiles[n][:, k * D + h * 512 : k * D + (h + 1) * 512],
                                start=(k == 0),
                                stop=(k == KD - 1),
                            )
                for n in conds:
                    e_sb = epool.tile([128, D], BF16, name="e_sb", tag="e_sb")
                    for h in range(2):
                        nc.vector.tensor_copy(
                            e_sb[:, h * 512 : (h + 1) * 512], e_ps[n][h][:]
                        )
                    nc.scalar.dma_start(
                        sends[g][:, n - N_OFF[g], :, :], e_sb[:].bitcast(F32)
                    )
                # collective doorbell for this group; recv loads issue
                # after every doorbell so the gpsimd FIFO never makes a
                # later collective wait on an earlier group's wire time
                nc.gpsimd.collective_compute(
                    "AllToAll",
                    mybir.AluOpType.bypass,
                    replica_groups=[list(range(NCORES))],
                    ins=[sends[g][:].opt()],
                    outs=[recvs[g][:].opt()],
                )
            for g, gs in enumerate(GS):
                rows = slice(R_OFF[g], R_OFF[g] + NCORES * gs)
                nc.scalar.dma_start(
                    r_sb[rows, :],
                    recvs[g][:].rearrange("a n b d -> (a n) (b d)").bitcast(BF16),
                )
        wstack.close()

        # ---- phase D: attention reduce + p-major bf16 output -------------
        with (
            tc.tile_pool(name="rpsum", bufs=1, space="PSUM") as rpsum,
            tc.tile_pool(name="spool", bufs=2) as spool,
        ):
            JC = 8  # j-chunks per output DMA (4 KB/partition)
            for jj in range(BL * 2 // JC):
                stg = spool.tile([P, JC * 512], BF16, name="stg", tag="stg")
                for j2 in range(JC):
                    j = jj * JC + j2
                    o_ps = rpsum.tile([P, 512], F32, name="o_ps", tag=f"o{j % 4}")
                    nc.tensor.matmul(
                        o_ps[:],
                        attT72[:],
                        r_sb[:, j * 512 : (j + 1) * 512],
                        start=True,
                        stop=True,
                    )
                    nc.vector.tensor_copy(
                        stg[:, j2 * 512 : (j2 + 1) * 512], o_ps[:]
                    )
                nc.scalar.dma_start(
                    out_cond[:, jj * JC * 512 : (jj + 1) * JC * 512], stg[:]
                )

    _split_multiwait_drains(nc)
    return nc


_NC_CACHE = {}
_LAST_IN_MAPS = None


def _get_nc():
    key = (WNAME, tuple(GROUPS))
    if key not in _NC_CACHE:
        _NC_CACHE[key] = _build()
    return _NC_CACHE[key]


def _host_att(W1, b1, W2, b2, cat_enc):
    h = np.maximum(cat_enc @ W1 + b1, 0.0)
    a = h @ W2 + b2
    a = a - a.max(axis=-1, keepdims=True)
    e = np.exp(a)
    return e / e.sum(axis=-1, keepdims=True)


def kernel(image, W_emb, b_emb, W_rep, b_rep, mask_table, W1, b1, W2, b2, cat_enc):
    import ml_dtypes

    f8 = ml_dtypes.float8_e4m3fn
    bf = ml_dtypes.bfloat16

    image = np.asarray(image, np.float32)
    W_emb = np.asarray(W_emb, np.float32)
    b_emb = np.asarray(b_emb, np.float32).reshape(1, D)
    W_rep = np.asarray(W_rep, np.float32)
    b_rep = np.asarray(b_rep, np.float32)
    mask_table = np.asarray(mask_table, np.float32)
    W1 = np.asarray(W1, np.float32)
    b1 = np.asarray(b1, np.float32).reshape(1, N)
    W2 = np.asarray(W2, np.float32)
    b2 = np.asarray(b2, np.float32).reshape(1, N)
    cat_enc = np.asarray(cat_enc, np.float32)

    # fold the mask (and fp8 scale) into the per-condition weights
    wm = W_rep * mask_table[:, None, :]          # [N, D, D] premasked
    if WSCALE != 1.0:
        wm = wm * WSCALE
    wrep_pad = np.zeros((NPAD, D, D), np.float32)
    wrep_pad[:N] = wm
    wrep_cast = wrep_pad.astype(f8 if WNAME == "fp8" else bf)

    # host-side layouts for the x GEMM
    img_t = np.ascontiguousarray(
        image.T.reshape(KF, 128, 128).transpose(1, 0, 2)
    ).astype(bf)                                  # [128, KF, 128]
    wemb_t = np.ascontiguousarray(
        W_emb.reshape(KF, 128, D).transpose(1, 0, 2)
    ).astype(bf)                                  # [128, KF, D]

    # R row r = R_OFF[g] + src*gs + gi holds condition 9*src + N_OFF[g] + gi
    GS = list(GROUPS)
    N_OFF = [sum(GS[:g]) for g in range(len(GS))]
    R_OFF = [NCORES * o for o in N_OFF]
    n_of_r = np.empty(NPAD, np.int64)
    for g in range(len(GS)):
        for src in range(NCORES):
            for gi in range(GS[g]):
                n_of_r[R_OFF[g] + src * GS[g] + gi] = NL * src + N_OFF[g] + gi
    psel = np.zeros((N, NPAD), np.float32)
    for r in range(NPAD):
        if n_of_r[r] < N:
            psel[n_of_r[r], r] = 1.0 / WSCALE

    nc = _get_nc()
    in_maps = []
    for i in range(NCORES):
        in_maps.append({
            "img_t": img_t,
            "wemb_t": wemb_t,
            "b_emb": b_emb.astype(bf),
            "w_rep_l": np.ascontiguousarray(wrep_cast[i * NL : (i + 1) * NL]),
            "w1": W1, "b1": b1, "w2": W2, "b2": b2,
            "cat_enc": cat_enc, "perm_sel": psel,
        })

    global _LAST_IN_MAPS
    _LAST_IN_MAPS = in_maps
    res = run_bass_kernel_spmd(nc, in_maps, list(range(NCORES)))

    out = np.empty((B, P + N, D), np.float32)
    for i in range(NCORES):
        oc = np.asarray(res.results[i]["out_cond"]).reshape(P, BL, D)
        out[i * BL : (i + 1) * BL, :P] = oc.transpose(1, 0, 2).astype(np.float32)
    x = np.asarray(res.results[0]["x_out"])
    out[:, P:] = x[:, None, :]

    # host correction for the (mask * b_rep) bias term (zero in practice)
    if np.any(b_rep):
        att = _host_att(W1, b1, W2, b2, cat_enc)
        corr = att @ (mask_table * b_rep)        # [P, D]
        out[:, :P] += corr[None]
    return out


# revision 17
# speedup vs baseline: 1.1401x; 1.0429x over previous
"""Trainium2 Bass kernel for ConditionalSimNet2 (moe_routing).

Computation (B=128, FEAT_IN=2048, D=1024, N=P=66 conditions):
    x          = image @ W_emb + b_emb                    [B, D]
    masked_rep = einsum('bd,nde->bne', x, W_rep) + b_rep  [B, N, D]
    embed      = mask_table * masked_rep                  [B, N, D]
    att        = softmax(relu(cat_enc@W1+b1)@W2 + b2)     [P, N]
    cond_feat  = einsum('pn,bnd->bpd', att, embed)        [B, P, D]
    out        = concat([cond_feat, broadcast(x)], 1)     [B, P+N, D]

Sharding: expert-parallel over the 66 conditions on 8 cores (9 each,
zero-padded to 72).  Every core computes x and att redundantly (cheap),
runs its 9 grouped GEMMs against its W_rep shard (the dominant HBM
traffic), exchanges embed slices with AllToAll so each core holds all
66 conditions for its 16-row batch shard, reduces with a single-K
matmul and writes its [66, 16*1024] output shard (p-major, bf16); the
host transposes/upcasts and broadcasts feature_x from core 0's x.

Key structural points vs the naive version:
  - mask_table (and the b_rep bias, via a host-side att-weighted
    correction) are folded into W_rep ON THE HOST, so the device never
    touches masks: no DVE broadcast-multiply, no bias matmuls in the
    hot loop.
  - image arrives pre-transposed/pre-cast (imgT bf16) and W_emb
    pre-cast bf16 k-major: the x GEMM is pure bf16 with no device-side
    image transposes.
  - grouped GEMM is k-outer over groups of conditions: the stationary
    operand (xT k-slice) is reused across the group, keeping the PE
    warm and LDWEIGHTS amortized.
  - DMA queues are specialized: sync ring = W_rep weight stream (plus
    two early W_emb chunks); scalar ring = startup loads + a2a sends +
    recv loads + output writes; gpsimd = collective doorbells only,
    issued back-to-back so a later collective is never queued behind
    an earlier group's wire time.
  - a2a payloads are bf16 PACKED AS F32 (collective cost scales with
    element count via 2048-elem CCE descriptors, not bytes) and
    chunked [1,4,4]: the tiny first group absorbs the entry barrier
    and ncfw first-op anomaly, the rest pipelines with the GEMM.
  - W_rep can optionally ship in fp8e4m3 (scaled by WSCALE, de-scaled
    through the attention matrix) to halve the dominant HBM stream.
"""

import os
import sys

import numpy as np

try:
    import concourse.bass as bass
except ImportError:  # pragma: no cover - fallback when PYTHONPATH is not set
    sys.path.insert(0, "/opt/trn_rl_repo")
    import concourse.bass as bass

import concourse.mybir as mybir
import concourse.tile as tile
from concourse.bass_utils import run_bass_kernel_spmd
from concourse.masks import make_identity

F32 = mybir.dt.float32
BF16 = mybir.dt.bfloat16
FP8 = mybir.dt.float8e4

B = 128          # batch
FI = 2048        # backbone feature dim
D = 1024         # embed dim
N = 66           # conditions (== pair categories P)
P = 66
CE = 24          # 2 * C_CAT
NCORES = 8
NL = 9           # conditions per core (66 -> 72 padded)
NPAD = NCORES * NL
BL = B // NCORES  # batch rows per core

KD = D // 128    # 8 k-tiles over D
KF = FI // 128   # 16 k-tiles over FEAT_IN

# weight dtype for the W_rep stream: bf16 (safe) or fp8 (half traffic)
WNAME = os.environ.get("CSN_WDT", "bf16")
WDT = FP8 if WNAME == "fp8" else BF16
WSCALE = float(os.environ.get("CSN_WSCALE", "128")) if WNAME == "fp8" else 1.0
# a2a group sizes (conditions per collective); first fires earliest
GROUPS = [int(x) for x in os.environ.get("CSN_GROUPS", "1,4,4").split(",")]
assert sum(GROUPS) == NL
# weight-pool prefetch depth (conditions)
WBUFS = int(os.environ.get("CSN_WBUFS", "8"))


def _split_multiwait_drains(nc):
    """This walrus build only accepts one sem wait per instruction; hoist
    extras onto NoOp carriers inserted just before the instruction (engines
    execute their stream in order, so wait-then-op is equivalent)."""
    fixno = 0
    for fnc in nc.m.functions:
        for bb in fnc.blocks:
            insts = bb.instructions
            i = 0
            while i < len(insts):
                inst = insts[i]
                si = inst.sync_info
                if si is not None and len(si.on_wait) > 1:
                    waits = list(si.on_wait)
                    si.on_wait = waits[-1:]
                    for w in waits[:-1]:
                        fixno += 1
                        carrier = mybir.InstNoOp(
                            name=f"I-waitfix-{fixno}",
                            engine=inst.engine,
                            ins=[],
                            outs=[],
                            sync_info=mybir.SyncInfo(on_wait=[w], on_update=[]),
                        )
                        insts.insert(i, carrier)
                        i += 1
                i += 1
    return fixno


def _build():
    nc = bass.Bass(
        "TRN2", target_bir_lowering=False, debug=False, num_devices=NCORES
    )
    ins = {
        "img_t": nc.dram_tensor("img_t", [128, KF, 128], BF16, kind="ExternalInput").ap(),
        "wemb_t": nc.dram_tensor("wemb_t", [128, KF, D], BF16, kind="ExternalInput").ap(),
        "b_emb": nc.dram_tensor("b_emb", [1, D], BF16, kind="ExternalInput").ap(),
        "w_rep_l": nc.dram_tensor("w_rep_l", [NL, D, D], WDT, kind="ExternalInput").ap(),
        "w1": nc.dram_tensor("w1", [CE, N], F32, kind="ExternalInput").ap(),
        "b1": nc.dram_tensor("b1", [1, N], F32, kind="ExternalInput").ap(),
        "w2": nc.dram_tensor("w2", [N, N], F32, kind="ExternalInput").ap(),
        "b2": nc.dram_tensor("b2", [1, N], F32, kind="ExternalInput").ap(),
        "cat_enc": nc.dram_tensor("cat_enc", [N, CE], F32, kind="ExternalInput").ap(),
        "perm_sel": nc.dram_tensor("perm_sel", [N, NPAD], F32, kind="ExternalInput").ap(),
    }
    out_cond = nc.dram_tensor(
        "out_cond", [P, BL * D], BF16, kind="ExternalOutput"
    ).ap()
    x_out = nc.dram_tensor("x_out", [B, D], F32, kind="ExternalOutput").ap()

    GS = list(GROUPS)
    N_OFF = [sum(GS[:g]) for g in range(len(GS))]
    R_OFF = [NCORES * o for o in N_OFF]
    # a2a payloads are bf16 packed into f32 words: collective time scales
    # with ELEMENT count (2048-elem CCE descriptors), not bytes
    sends = [
        nc.dram_tensor(f"a2a_send{g}", [NCORES, gs, BL, D // 2], F32)
        for g, gs in enumerate(GS)
    ]
    recvs = [
        nc.dram_tensor(f"a2a_recv{g}", [NCORES, gs, BL, D // 2], F32)
        for g, gs in enumerate(GS)
    ]

    with tile.TileContext(nc) as tc, tc.tile_pool(name="const", bufs=1) as cpool:
        id_sb = cpool.tile([128, 128], F32, name="id_sb")
        make_identity(nc, id_sb[:])

        # ---- pool scopes (LIFO): wpool outermost, startup pools inner ----
        from contextlib import ExitStack

        wstack = ExitStack()
        wpool = wstack.enter_context(tc.tile_pool(name="wpool", bufs=WBUFS))
        bstack = ExitStack()
        bpool = bstack.enter_context(tc.tile_pool(name="bpool", bufs=1))
        wembp = bstack.enter_context(tc.tile_pool(name="wembp", bufs=3))
        imgT_sb = bpool.tile([128, KF * 128], BF16, name="imgT_sb")
        nc.scalar.dma_start(
            imgT_sb[:].rearrange("p (t b) -> p t b", t=KF), ins["img_t"][:]
        )
        WEC = 4  # W_emb chunks of 4 k-tiles (1 MB each)
        wemb_ch = []
        for c in range(WEC):
            wc = wembp.tile([128, 4 * D], BF16, name="wemb", tag="wemb")
            eng = nc.sync if c < 2 else nc.scalar
            eng.dma_start(
                wc[:].rearrange("p (t d) -> p t d", t=4),
                ins["wemb_t"][:, c * 4 : (c + 1) * 4, :],
            )
            wemb_ch.append(wc)

        # tiny attention inputs (behind the startup loads; att is not urgent)
        ce_sb = cpool.tile([N, CE], F32, name="ce_sb")
        nc.scalar.dma_start(ce_sb[:], ins["cat_enc"][:])
        w1_sb = cpool.tile([CE, N], F32, name="w1_sb")
        nc.scalar.dma_start(w1_sb[:], ins["w1"][:])
        b1_sb = cpool.tile([1, N], F32, name="b1_sb")
        nc.scalar.dma_start(b1_sb[:], ins["b1"][:])
        w2_sb = cpool.tile([N, N], F32, name="w2_sb")
        nc.scalar.dma_start(w2_sb[:], ins["w2"][:])
        b2_sb = cpool.tile([1, N], F32, name="b2_sb")
        nc.scalar.dma_start(b2_sb[:], ins["b2"][:])
        psel_sb = cpool.tile([N, NPAD], F32, name="psel_sb")
        nc.scalar.dma_start(psel_sb[:], ins["perm_sel"][:])
        bemb_sb = cpool.tile([1, D], BF16, name="bemb_sb")
        nc.scalar.dma_start(bemb_sb[:], ins["b_emb"][:])

        # ---- W_rep weight stream: sync ring only, one 2 MB DMA / cond ----
        wtiles = []
        for n in range(NL):
            wt = wpool.tile([128, KD * D], WDT, name="wt", tag="wt")
            nc.sync.dma_start(
                wt[:].rearrange("p (k d) -> p k d", k=KD),
                ins["w_rep_l"][n].rearrange("(k p) d -> p k d", p=128),
            )
            wtiles.append(wt)

        # ---- ones row (f32 for att bias folds, bf16 for b_emb fold) ------
        onesA_sb = cpool.tile([1, 128], F32, name="onesA_sb")
        nc.vector.memset(onesA_sb[:], 1.0)
        ones_sb = cpool.tile([1, 128], BF16, name="ones_sb")
        nc.vector.tensor_copy(ones_sb[:], onesA_sb[:])

        # ---- phase A: attention matrix -> attT72 [NPAD, P] bf16 ----------
        attT72 = cpool.tile([NPAD, P], BF16, name="attT72")
        with tc.tile_pool(name="attp", bufs=1, space="PSUM") as attp:
            ceT_ps = attp.tile([CE, N], F32, name="ceT_ps")
            nc.tensor.transpose(ceT_ps[:], ce_sb[:], id_sb[:N, :N])
            ceT_sb = cpool.tile([CE, N], F32, name="ceT_sb")
            nc.vector.tensor_copy(ceT_sb[:], ceT_ps[:])

            h_ps = attp.tile([P, N], F32, name="h_ps")
            nc.tensor.matmul(h_ps[:], ceT_sb[:], w1_sb[:], start=True, stop=False)
            nc.tensor.matmul(h_ps[:], onesA_sb[:, :P], b1_sb[:], start=False, stop=True)
            h_sb = cpool.tile([P, N], F32, name="h_sb")
            nc.scalar.activation(h_sb[:], h_ps[:], mybir.ActivationFunctionType.Relu)

            hT_ps = attp.tile([N, P], F32, name="hT_ps")
            nc.tensor.transpose(hT_ps[:], h_sb[:], id_sb[:P, :P])
            hT_sb = cpool.tile([N, P], F32, name="hT_sb")
            nc.vector.tensor_copy(hT_sb[:], hT_ps[:])

            a_ps = attp.tile([P, N], F32, name="a_ps")
            nc.tensor.matmul(a_ps[:], hT_sb[:], w2_sb[:], start=True, stop=False)
            nc.tensor.matmul(a_ps[:], onesA_sb[:, :P], b2_sb[:], start=False, stop=True)
            att_sb = cpool.tile([P, N], F32, name="att_sb")
            nc.vector.tensor_copy(att_sb[:], a_ps[:])

            # row softmax
            rmax = cpool.tile([P, 1], F32, name="rmax")
            nc.vector.tensor_reduce(
                rmax[:], att_sb[:], axis=mybir.AxisListType.X, op=mybir.AluOpType.max
            )
            nc.vector.tensor_scalar_mul(rmax[:], rmax[:], -1.0)
            rsum = cpool.tile([P, 1], F32, name="rsum")
            nc.scalar.activation(
                att_sb[:],
                att_sb[:],
                mybir.ActivationFunctionType.Exp,
                bias=rmax[:],
                accum_out=rsum[:],
            )
            nc.vector.reciprocal(rsum[:], rsum[:])
            nc.vector.tensor_scalar_mul(att_sb[:], att_sb[:], rsum[:])

            attT_ps = attp.tile([N, P], F32, name="attT_ps")
            nc.tensor.transpose(attT_ps[:], att_sb[:], id_sb[:P, :P])
            attT_sb = cpool.tile([N, P], F32, name="attT_sb")
            nc.vector.tensor_copy(attT_sb[:], attT_ps[:])

            # permute att rows into R order (and fold 1/WSCALE, baked into
            # perm_sel on the host)
            attT72_ps = attp.tile([NPAD, P], F32, name="attT72_ps")
            nc.tensor.matmul(
                attT72_ps[:], psel_sb[:], attT_sb[:], start=True, stop=True
            )
            nc.vector.tensor_copy(attT72[:], attT72_ps[:])

        # ---- phase B: x = imgT.T @ W_emb + b_emb, then xT ---------------
        x_sb = cpool.tile([128, D], F32, name="x_sb")
        xT_sb = cpool.tile([128, D], BF16, name="xT_sb")  # 8 blocks [128d,128b]
        with (
            tc.tile_pool(name="bpsum", bufs=2, space="PSUM") as bpsum,
            tc.tile_pool(name="tpsum", bufs=2, space="PSUM") as tpsum,
        ):
            x_ps = [bpsum.tile([128, 512], F32, name=f"x_ps{h}") for h in range(2)]
            for k in range(KF):
                wc = wemb_ch[k // 4]
                kk = k % 4
                for h in range(2):
                    nc.tensor.matmul(
                        x_ps[h][:],
                        imgT_sb[:, k * 128 : (k + 1) * 128],
                        wc[:, kk * D + h * 512 : kk * D + (h + 1) * 512],
                        start=(k == 0),
                        stop=False,
                    )
            for h in range(2):
                nc.tensor.matmul(
                    x_ps[h][:],
                    ones_sb[:],
                    bemb_sb[:, h * 512 : (h + 1) * 512],
                    start=False,
                    stop=True,
                )
                nc.vector.tensor_copy(x_sb[:, h * 512 : (h + 1) * 512], x_ps[h][:])
            nc.scalar.dma_start(x_out[:], x_sb[:])
            for m in range(KD):
                tp = tpsum.tile([128, 128], F32, name="tp", tag="tp")
                nc.tensor.transpose(tp[:], x_sb[:, m * 128 : (m + 1) * 128], id_sb[:])
                nc.vector.tensor_copy(xT_sb[:, m * 128 : (m + 1) * 128], tp[:])
        bstack.close()

        # ---- phase C: grouped GEMM (k-outer per group) + a2a pipeline ----
        r_sb = cpool.tile([NPAD, BL * D], BF16, name="r_sb")
        with (
            tc.tile_pool(name="epool", bufs=3) as epool,
            tc.tile_pool(name="cpsum", bufs=1, space="PSUM") as cpsum,
        ):
            for g, gs in enumerate(GS):
                conds = list(range(N_OFF[g], N_OFF[g] + gs))
                e_ps = {
                    n: [
                        cpsum.tile(
                            [128, 512], F32, name="e_ps", tag=f"e{n % 4}_{h}"
                        )
                        for h in range(2)
                    ]
                    for n in conds
                }
                for k in range(KD):
                    for n in conds:
                        for h in range(2):
                            nc.tensor.matmul(
                                e_ps[n][h][:],
                                xT_sb[:, k * 128 : (k + 1) * 128],
                                wtiles[n][:, k * D + h * 512 : k * D + (h + 1) * 512],
                                start=(k == 0),
                                stop=(k == KD - 1),
                            )
                for n in conds:
                    e_sb = epool.tile([128, D], BF16, name="e_sb", tag="e_sb")
                    # drain the two PSUM halves on two engines in parallel
                    nc.vector.tensor_copy(e_sb[:, 0:512], e_ps[n][0][:])
                    nc.scalar.activation(
                        e_sb[:, 512:D],
                        e_ps[n][1][:],
                        mybir.ActivationFunctionType.Copy,
                    )
                    nc.scalar.dma_start(
                        sends[g][:, n - N_OFF[g], :, :], e_sb[:].bitcast(F32)
                    )
                # collective doorbell for this group; recv loads issue
                # after every doorbell so the gpsimd FIFO never makes a
                # later collective wait on an earlier group's wire time
                nc.gpsimd.collective_compute(
                    "AllToAll",
                    mybir.AluOpType.bypass,
                    replica_groups=[list(range(NCORES))],
                    ins=[sends[g][:].opt()],
                    outs=[recvs[g][:].opt()],
                )
            for g, gs in enumerate(GS):
                rows = slice(R_OFF[g], R_OFF[g] + NCORES * gs)
                nc.scalar.dma_start(
                    r_sb[rows, :],
                    recvs[g][:].rearrange("a n b d -> (a n) (b d)").bitcast(BF16),
                )
        wstack.close()

        # ---- phase D: attention reduce + p-major bf16 output -------------
        with (
            tc.tile_pool(name="rpsum", bufs=1, space="PSUM") as rpsum,
            tc.tile_pool(name="spool", bufs=2) as spool,
        ):
            JC = 8  # j-chunks per output DMA (4 KB/partition)
            for jj in range(BL * 2 // JC):
                stg = spool.tile([P, JC * 512], BF16, name="stg", tag="stg")
                for j2 in range(JC):
                    j = jj * JC + j2
                    o_ps = rpsum.tile([P, 512], F32, name="o_ps", tag=f"o{j % 8}")
                    nc.tensor.matmul(
                        o_ps[:],
                        attT72[:],
                        r_sb[:, j * 512 : (j + 1) * 512],
                        start=True,
                        stop=True,
                    )
                    # the reduce tail is drain-bound: alternate the PSUM->SBUF
                    # casts between DVE and ACT so neither engine serializes it
                    dst = stg[:, j2 * 512 : (j2 + 1) * 512]
                    if j % 2 == 0:
                        nc.vector.tensor_copy(dst, o_ps[:])
                    else:
                        nc.scalar.activation(
                            dst, o_ps[:], mybir.ActivationFunctionType.Copy
                        )
                # output writes ride the sync ring, idle after the weight
                # stream ends, keeping ACT free for the casts
                nc.sync.dma_start(
                    out_cond[:, jj * JC * 512 : (jj + 1) * JC * 512], stg[:]
                )

    _split_multiwait_drains(nc)
    return nc


_NC_CACHE = {}
_LAST_IN_MAPS = None


def _get_nc():
    key = (WNAME, tuple(GROUPS))
    if key not in _NC_CACHE:
        _NC_CACHE[key] = _build()
    return _NC_CACHE[key]


def _host_att(W1, b1, W2, b2, cat_enc):
    h = np.maximum(cat_enc @ W1 + b1, 0.0)
    a = h @ W2 + b2
    a = a - a.max(axis=-1, keepdims=True)
    e = np.exp(a)
    return e / e.sum(axis=-1, keepdims=True)


def kernel(image, W_emb, b_emb, W_rep, b_rep, mask_table, W1, b1, W2, b2, cat_enc):
    import ml_dtypes

    f8 = ml_dtypes.float8_e4m3fn
    bf = ml_dtypes.bfloat16

    image = np.asarray(image, np.float32)
    W_emb = np.asarray(W_emb, np.float32)
    b_emb = np.asarray(b_emb, np.float32).reshape(1, D)
    W_rep = np.asarray(W_rep, np.float32)
    b_rep = np.asarray(b_rep, np.float32)
    mask_table = np.asarray(mask_table, np.float32)
    W1 = np.asarray(W1, np.float32)
    b1 = np.asarray(b1, np.float32).reshape(1, N)
    W2 = np.asarray(W2, np.float32)
    b2 = np.asarray(b2, np.float32).reshape(1, N)
    cat_enc = np.asarray(cat_enc, np.float32)

    # fold the mask (and fp8 scale) into the per-condition weights
    wm = W_rep * mask_table[:, None, :]          # [N, D, D] premasked
    if WSCALE != 1.0:
        wm = wm * WSCALE
    wrep_pad = np.zeros((NPAD, D, D), np.float32)
    wrep_pad[:N] = wm
    wrep_cast = wrep_pad.astype(f8 if WNAME == "fp8" else bf)

    # host-side layouts for the x GEMM
    img_t = np.ascontiguousarray(
        image.T.reshape(KF, 128, 128).transpose(1, 0, 2)
    ).astype(bf)                                  # [128, KF, 128]
    wemb_t = np.ascontiguousarray(
        W_emb.reshape(KF, 128, D).transpose(1, 0, 2)
    ).astype(bf)                                  # [128, KF, D]

    # R row r = R_OFF[g] + src*gs + gi holds condition 9*src + N_OFF[g] + gi
    GS = list(GROUPS)
    N_OFF = [sum(GS[:g]) for g in range(len(GS))]
    R_OFF = [NCORES * o for o in N_OFF]
    n_of_r = np.empty(NPAD, np.int64)
    for g in range(len(GS)):
        for src in range(NCORES):
            for gi in range(GS[g]):
                n_of_r[R_OFF[g] + src * GS[g] + gi] = NL * src + N_OFF[g] + gi
    psel = np.zeros((N, NPAD), np.float32)
    for r in range(NPAD):
        if n_of_r[r] < N:
            psel[n_of_r[r], r] = 1.0 / WSCALE

    nc = _get_nc()
    in_maps = []
    for i in range(NCORES):
        in_maps.append({
            "img_t": img_t,
            "wemb_t": wemb_t,
            "b_emb": b_emb.astype(bf),
            "w_rep_l": np.ascontiguousarray(wrep_cast[i * NL : (i + 1) * NL]),
            "w1": W1, "b1": b1, "w2": W2, "b2": b2,
            "cat_enc": cat_enc, "perm_sel": psel,
        })

    global _LAST_IN_MAPS
    _LAST_IN_MAPS = in_maps
    res = run_bass_kernel_spmd(nc, in_maps, list(range(NCORES)))

    out = np.empty((B, P + N, D), np.float32)
    for i in range(NCORES):
        oc = np.asarray(res.results[i]["out_cond"]).reshape(P, BL, D)
        out[i * BL : (i + 1) * BL, :P] = oc.transpose(1, 0, 2).astype(np.float32)
    x = np.asarray(res.results[0]["x_out"])
    out[:, P:] = x[:, None, :]

    # host correction for the (mask * b_rep) bias term (zero in practice)
    if np.any(b_rep):
        att = _host_att(W1, b1, W2, b2, cat_enc)
        corr = att @ (mask_table * b_rep)        # [P, D]
        out[:, :P] += corr[None]
    return out


# revision 18
# speedup vs baseline: 1.1605x; 1.0180x over previous
"""Trainium2 Bass kernel for ConditionalSimNet2 (moe_routing).

Computation (B=128, FEAT_IN=2048, D=1024, N=P=66 conditions):
    x          = image @ W_emb + b_emb                    [B, D]
    masked_rep = einsum('bd,nde->bne', x, W_rep) + b_rep  [B, N, D]
    embed      = mask_table * masked_rep                  [B, N, D]
    att        = softmax(relu(cat_enc@W1+b1)@W2 + b2)     [P, N]
    cond_feat  = einsum('pn,bnd->bpd', att, embed)        [B, P, D]
    out        = concat([cond_feat, broadcast(x)], 1)     [B, P+N, D]

Sharding: expert-parallel over the 66 conditions on 8 cores (9 each,
zero-padded to 72).  Every core computes x and att redundantly (cheap),
runs its 9 grouped GEMMs against its W_rep shard (the dominant HBM
traffic), exchanges embed slices with AllToAll so each core holds all
66 conditions for its 16-row batch shard, reduces with a single-K
matmul and writes its [66, 16*1024] output shard (p-major, bf16); the
host transposes/upcasts and broadcasts feature_x from core 0's x.

Key structural points vs the naive version:
  - mask_table (and the b_rep bias, via a host-side att-weighted
    correction) are folded into W_rep ON THE HOST, so the device never
    touches masks: no DVE broadcast-multiply, no bias matmuls in the
    hot loop.
  - image arrives pre-transposed/pre-cast (imgT bf16) and W_emb
    pre-cast bf16 k-major: the x GEMM is pure bf16 with no device-side
    image transposes.
  - grouped GEMM is k-outer over groups of conditions: the stationary
    operand (xT k-slice) is reused across the group, keeping the PE
    warm and LDWEIGHTS amortized.
  - DMA queues are specialized: sync ring = W_rep weight stream (plus
    two early W_emb chunks); scalar ring = startup loads + a2a sends +
    recv loads + output writes; gpsimd = collective doorbells only,
    issued back-to-back so a later collective is never queued behind
    an earlier group's wire time.
  - a2a payloads are bf16 PACKED AS F32 (collective cost scales with
    element count via 2048-elem CCE descriptors, not bytes) and
    chunked [1,4,4]: the tiny first group absorbs the entry barrier
    and ncfw first-op anomaly, the rest pipelines with the GEMM.
  - W_rep can optionally ship in fp8e4m3 (scaled by WSCALE, de-scaled
    through the attention matrix) to halve the dominant HBM stream.
"""

import os
import sys

import numpy as np

try:
    import concourse.bass as bass
except ImportError:  # pragma: no cover - fallback when PYTHONPATH is not set
    sys.path.insert(0, "/opt/trn_rl_repo")
    import concourse.bass as bass

import concourse.mybir as mybir
import concourse.tile as tile
from concourse.bass_utils import run_bass_kernel_spmd
from concourse.masks import make_identity

F32 = mybir.dt.float32
BF16 = mybir.dt.bfloat16
FP8 = mybir.dt.float8e4

B = 128          # batch
FI = 2048        # backbone feature dim
D = 1024         # embed dim
N = 66           # conditions (== pair categories P)
P = 66
CE = 24          # 2 * C_CAT
NCORES = 8
NL = 9           # conditions per core (66 -> 72 padded)
NPAD = NCORES * NL
BL = B // NCORES  # batch rows per core

KD = D // 128    # 8 k-tiles over D
KF = FI // 128   # 16 k-tiles over FEAT_IN

# weight dtype for the W_rep stream: bf16 (safe) or fp8 (half traffic)
WNAME = os.environ.get("CSN_WDT", "bf16")
WDT = FP8 if WNAME == "fp8" else BF16
WSCALE = float(os.environ.get("CSN_WSCALE", "128")) if WNAME == "fp8" else 1.0
# exchange dtype for the embed a2a: fp8 quarters the collective element
# count (packed into f32 words); embeds are scaled by ES (folded into the
# weights, de-scaled through perm_sel) to sit in e4m3 range
ENAME = os.environ.get("CSN_EDT", "fp8")
EDT = FP8 if ENAME == "fp8" else BF16
EPACK = 4 if ENAME == "fp8" else 2
ESCALE = 32.0 if ENAME == "fp8" else 1.0
# a2a group sizes (conditions per collective); first fires earliest
GROUPS = [int(x) for x in os.environ.get("CSN_GROUPS", "1,4,4").split(",")]
assert sum(GROUPS) == NL
# weight-pool prefetch depth (conditions)
WBUFS = int(os.environ.get("CSN_WBUFS", "8"))


def _split_multiwait_drains(nc):
    """This walrus build only accepts one sem wait per instruction; hoist
    extras onto NoOp carriers inserted just before the instruction (engines
    execute their stream in order, so wait-then-op is equivalent)."""
    fixno = 0
    for fnc in nc.m.functions:
        for bb in fnc.blocks:
            insts = bb.instructions
            i = 0
            while i < len(insts):
                inst = insts[i]
                si = inst.sync_info
                if si is not None and len(si.on_wait) > 1:
                    waits = list(si.on_wait)
                    si.on_wait = waits[-1:]
                    for w in waits[:-1]:
                        fixno += 1
                        carrier = mybir.InstNoOp(
                            name=f"I-waitfix-{fixno}",
                            engine=inst.engine,
                            ins=[],
                            outs=[],
                            sync_info=mybir.SyncInfo(on_wait=[w], on_update=[]),
                        )
                        insts.insert(i, carrier)
                        i += 1
                i += 1
    return fixno


def _build():
    nc = bass.Bass(
        "TRN2", target_bir_lowering=False, debug=False, num_devices=NCORES
    )
    ins = {
        "img_t": nc.dram_tensor("img_t", [128, KF, 128], BF16, kind="ExternalInput").ap(),
        "wemb_t": nc.dram_tensor("wemb_t", [128, KF, D], BF16, kind="ExternalInput").ap(),
        "b_emb": nc.dram_tensor("b_emb", [1, D], BF16, kind="ExternalInput").ap(),
        "w_rep_l": nc.dram_tensor("w_rep_l", [NL, D, D], WDT, kind="ExternalInput").ap(),
        "w1": nc.dram_tensor("w1", [CE, N], F32, kind="ExternalInput").ap(),
        "b1": nc.dram_tensor("b1", [1, N], F32, kind="ExternalInput").ap(),
        "w2": nc.dram_tensor("w2", [N, N], F32, kind="ExternalInput").ap(),
        "b2": nc.dram_tensor("b2", [1, N], F32, kind="ExternalInput").ap(),
        "cat_enc": nc.dram_tensor("cat_enc", [N, CE], F32, kind="ExternalInput").ap(),
        "perm_sel": nc.dram_tensor("perm_sel", [N, NPAD], F32, kind="ExternalInput").ap(),
    }
    out_cond = nc.dram_tensor(
        "out_cond", [P, BL * D], BF16, kind="ExternalOutput"
    ).ap()
    x_out = nc.dram_tensor("x_out", [B, D], F32, kind="ExternalOutput").ap()

    GS = list(GROUPS)
    N_OFF = [sum(GS[:g]) for g in range(len(GS))]
    R_OFF = [NCORES * o for o in N_OFF]
    # a2a payloads are bf16 packed into f32 words: collective time scales
    # with ELEMENT count (2048-elem CCE descriptors), not bytes
    sends = [
        nc.dram_tensor(f"a2a_send{g}", [NCORES, gs, BL, D // EPACK], F32)
        for g, gs in enumerate(GS)
    ]
    recvs = [
        nc.dram_tensor(f"a2a_recv{g}", [NCORES, gs, BL, D // EPACK], F32)
        for g, gs in enumerate(GS)
    ]

    with tile.TileContext(nc) as tc, tc.tile_pool(name="const", bufs=1) as cpool:
        id_sb = cpool.tile([128, 128], F32, name="id_sb")
        make_identity(nc, id_sb[:])

        # ---- pool scopes (LIFO): wpool outermost, startup pools inner ----
        from contextlib import ExitStack

        wstack = ExitStack()
        wpool = wstack.enter_context(tc.tile_pool(name="wpool", bufs=WBUFS))
        bstack = ExitStack()
        bpool = bstack.enter_context(tc.tile_pool(name="bpool", bufs=1))
        wembp = bstack.enter_context(tc.tile_pool(name="wembp", bufs=3))
        imgT_sb = bpool.tile([128, KF * 128], BF16, name="imgT_sb")
        nc.scalar.dma_start(
            imgT_sb[:].rearrange("p (t b) -> p t b", t=KF), ins["img_t"][:]
        )
        WEC = 4  # W_emb chunks of 4 k-tiles (1 MB each)
        wemb_ch = []
        for c in range(WEC):
            wc = wembp.tile([128, 4 * D], BF16, name="wemb", tag="wemb")
            eng = nc.sync if c < 2 else nc.scalar
            eng.dma_start(
                wc[:].rearrange("p (t d) -> p t d", t=4),
                ins["wemb_t"][:, c * 4 : (c + 1) * 4, :],
            )
            wemb_ch.append(wc)

        # tiny attention inputs (behind the startup loads; att is not urgent)
        ce_sb = cpool.tile([N, CE], F32, name="ce_sb")
        nc.scalar.dma_start(ce_sb[:], ins["cat_enc"][:])
        w1_sb = cpool.tile([CE, N], F32, name="w1_sb")
        nc.scalar.dma_start(w1_sb[:], ins["w1"][:])
        b1_sb = cpool.tile([1, N], F32, name="b1_sb")
        nc.scalar.dma_start(b1_sb[:], ins["b1"][:])
        w2_sb = cpool.tile([N, N], F32, name="w2_sb")
        nc.scalar.dma_start(w2_sb[:], ins["w2"][:])
        b2_sb = cpool.tile([1, N], F32, name="b2_sb")
        nc.scalar.dma_start(b2_sb[:], ins["b2"][:])
        psel_sb = cpool.tile([N, NPAD], F32, name="psel_sb")
        nc.scalar.dma_start(psel_sb[:], ins["perm_sel"][:])
        bemb_sb = cpool.tile([1, D], BF16, name="bemb_sb")
        nc.scalar.dma_start(bemb_sb[:], ins["b_emb"][:])

        # ---- W_rep weight stream: sync ring only, one 2 MB DMA / cond ----
        wtiles = []
        for n in range(NL):
            wt = wpool.tile([128, KD * D], WDT, name="wt", tag="wt")
            nc.sync.dma_start(
                wt[:].rearrange("p (k d) -> p k d", k=KD),
                ins["w_rep_l"][n].rearrange("(k p) d -> p k d", p=128),
            )
            wtiles.append(wt)

        # ---- ones row (f32 for att bias folds, bf16 for b_emb fold) ------
        onesA_sb = cpool.tile([1, 128], F32, name="onesA_sb")
        nc.vector.memset(onesA_sb[:], 1.0)
        ones_sb = cpool.tile([1, 128], BF16, name="ones_sb")
        nc.vector.tensor_copy(ones_sb[:], onesA_sb[:])

        # ---- phase A: attention matrix -> attT72 [NPAD, P] bf16 ----------
        attT72 = cpool.tile([NPAD, P], BF16, name="attT72")
        with tc.tile_pool(name="attp", bufs=1, space="PSUM") as attp:
            ceT_ps = attp.tile([CE, N], F32, name="ceT_ps")
            nc.tensor.transpose(ceT_ps[:], ce_sb[:], id_sb[:N, :N])
            ceT_sb = cpool.tile([CE, N], F32, name="ceT_sb")
            nc.vector.tensor_copy(ceT_sb[:], ceT_ps[:])

            h_ps = attp.tile([P, N], F32, name="h_ps")
            nc.tensor.matmul(h_ps[:], ceT_sb[:], w1_sb[:], start=True, stop=False)
            nc.tensor.matmul(h_ps[:], onesA_sb[:, :P], b1_sb[:], start=False, stop=True)
            h_sb = cpool.tile([P, N], F32, name="h_sb")
            nc.scalar.activation(h_sb[:], h_ps[:], mybir.ActivationFunctionType.Relu)

            hT_ps = attp.tile([N, P], F32, name="hT_ps")
            nc.tensor.transpose(hT_ps[:], h_sb[:], id_sb[:P, :P])
            hT_sb = cpool.tile([N, P], F32, name="hT_sb")
            nc.vector.tensor_copy(hT_sb[:], hT_ps[:])

            a_ps = attp.tile([P, N], F32, name="a_ps")
            nc.tensor.matmul(a_ps[:], hT_sb[:], w2_sb[:], start=True, stop=False)
            nc.tensor.matmul(a_ps[:], onesA_sb[:, :P], b2_sb[:], start=False, stop=True)
            att_sb = cpool.tile([P, N], F32, name="att_sb")
            nc.vector.tensor_copy(att_sb[:], a_ps[:])

            # row softmax
            rmax = cpool.tile([P, 1], F32, name="rmax")
            nc.vector.tensor_reduce(
                rmax[:], att_sb[:], axis=mybir.AxisListType.X, op=mybir.AluOpType.max
            )
            nc.vector.tensor_scalar_mul(rmax[:], rmax[:], -1.0)
            rsum = cpool.tile([P, 1], F32, name="rsum")
            nc.scalar.activation(
                att_sb[:],
                att_sb[:],
                mybir.ActivationFunctionType.Exp,
                bias=rmax[:],
                accum_out=rsum[:],
            )
            nc.vector.reciprocal(rsum[:], rsum[:])
            nc.vector.tensor_scalar_mul(att_sb[:], att_sb[:], rsum[:])

            attT_ps = attp.tile([N, P], F32, name="attT_ps")
            nc.tensor.transpose(attT_ps[:], att_sb[:], id_sb[:P, :P])
            attT_sb = cpool.tile([N, P], F32, name="attT_sb")
            nc.vector.tensor_copy(attT_sb[:], attT_ps[:])

            # permute att rows into R order (and fold 1/WSCALE, baked into
            # perm_sel on the host)
            attT72_ps = attp.tile([NPAD, P], F32, name="attT72_ps")
            nc.tensor.matmul(
                attT72_ps[:], psel_sb[:], attT_sb[:], start=True, stop=True
            )
            nc.vector.tensor_copy(attT72[:], attT72_ps[:])

        # ---- phase B: x = imgT.T @ W_emb + b_emb, then xT ---------------
        x_sb = cpool.tile([128, D], F32, name="x_sb")
        xT_sb = cpool.tile([128, D], BF16, name="xT_sb")  # 8 blocks [128d,128b]
        with (
            tc.tile_pool(name="bpsum", bufs=2, space="PSUM") as bpsum,
            tc.tile_pool(name="tpsum", bufs=2, space="PSUM") as tpsum,
        ):
            x_ps = [bpsum.tile([128, 512], F32, name=f"x_ps{h}") for h in range(2)]
            for k in range(KF):
                wc = wemb_ch[k // 4]
                kk = k % 4
                for h in range(2):
                    nc.tensor.matmul(
                        x_ps[h][:],
                        imgT_sb[:, k * 128 : (k + 1) * 128],
                        wc[:, kk * D + h * 512 : kk * D + (h + 1) * 512],
                        start=(k == 0),
                        stop=False,
                    )
            for h in range(2):
                nc.tensor.matmul(
                    x_ps[h][:],
                    ones_sb[:],
                    bemb_sb[:, h * 512 : (h + 1) * 512],
                    start=False,
                    stop=True,
                )
                nc.vector.tensor_copy(x_sb[:, h * 512 : (h + 1) * 512], x_ps[h][:])
            nc.scalar.dma_start(x_out[:], x_sb[:])
            for m in range(KD):
                tp = tpsum.tile([128, 128], F32, name="tp", tag="tp")
                nc.tensor.transpose(tp[:], x_sb[:, m * 128 : (m + 1) * 128], id_sb[:])
                nc.vector.tensor_copy(xT_sb[:, m * 128 : (m + 1) * 128], tp[:])
        bstack.close()

        # ---- phase C: grouped GEMM (k-outer per group) + a2a pipeline ----
        r_sb = cpool.tile([NPAD, BL * D], EDT, name="r_sb")
        with (
            tc.tile_pool(name="epool", bufs=3) as epool,
            tc.tile_pool(name="cpsum", bufs=1, space="PSUM") as cpsum,
        ):
            for g, gs in enumerate(GS):
                conds = list(range(N_OFF[g], N_OFF[g] + gs))
                e_ps = {
                    n: [
                        cpsum.tile(
                            [128, 512], F32, name="e_ps", tag=f"e{n % 4}_{h}"
                        )
                        for h in range(2)
                    ]
                    for n in conds
                }
                for k in range(KD):
                    for n in conds:
                        for h in range(2):
                            nc.tensor.matmul(
                                e_ps[n][h][:],
                                xT_sb[:, k * 128 : (k + 1) * 128],
                                wtiles[n][:, k * D + h * 512 : k * D + (h + 1) * 512],
                                start=(k == 0),
                                stop=(k == KD - 1),
                            )
                for n in conds:
                    e_sb = epool.tile([128, D], EDT, name="e_sb", tag="e_sb")
                    # drain the two PSUM halves on two engines in parallel
                    nc.vector.tensor_copy(e_sb[:, 0:512], e_ps[n][0][:])
                    nc.scalar.activation(
                        e_sb[:, 512:D],
                        e_ps[n][1][:],
                        mybir.ActivationFunctionType.Copy,
                    )
                    nc.scalar.dma_start(
                        sends[g][:, n - N_OFF[g], :, :], e_sb[:].bitcast(F32)
                    )
                # collective doorbell for this group; recv loads issue
                # after every doorbell so the gpsimd FIFO never makes a
                # later collective wait on an earlier group's wire time
                nc.gpsimd.collective_compute(
                    "AllToAll",
                    mybir.AluOpType.bypass,
                    replica_groups=[list(range(NCORES))],
                    ins=[sends[g][:].opt()],
                    outs=[recvs[g][:].opt()],
                )
            for g, gs in enumerate(GS):
                rows = slice(R_OFF[g], R_OFF[g] + NCORES * gs)
                nc.scalar.dma_start(
                    r_sb[rows, :],
                    recvs[g][:].rearrange("a n b d -> (a n) (b d)").bitcast(EDT),
                )
        wstack.close()

        # ---- phase D: attention reduce + p-major bf16 output -------------
        with (
            tc.tile_pool(name="rpsum", bufs=1, space="PSUM") as rpsum,
            tc.tile_pool(name="spool", bufs=2) as spool,
        ):
            JC = 8  # j-chunks per output DMA (4 KB/partition)
            for jj in range(BL * 2 // JC):
                stg = spool.tile([P, JC * 512], BF16, name="stg", tag="stg")
                for j2 in range(JC):
                    j = jj * JC + j2
                    o_ps = rpsum.tile([P, 512], F32, name="o_ps", tag=f"o{j % 8}")
                    nc.tensor.matmul(
                        o_ps[:],
                        attT72[:],
                        r_sb[:, j * 512 : (j + 1) * 512],
                        start=True,
                        stop=True,
                    )
                    # the reduce tail is drain-bound: alternate the PSUM->SBUF
                    # casts between DVE and ACT so neither engine serializes it
                    dst = stg[:, j2 * 512 : (j2 + 1) * 512]
                    if j % 2 == 0:
                        nc.vector.tensor_copy(dst, o_ps[:])
                    else:
                        nc.scalar.activation(
                            dst, o_ps[:], mybir.ActivationFunctionType.Copy
                        )
                # output writes ride the sync ring, idle after the weight
                # stream ends, keeping ACT free for the casts
                nc.sync.dma_start(
                    out_cond[:, jj * JC * 512 : (jj + 1) * JC * 512], stg[:]
                )

    _split_multiwait_drains(nc)
    return nc


_NC_CACHE = {}
_LAST_IN_MAPS = None


def _get_nc():
    key = (WNAME, ENAME, tuple(GROUPS))
    if key not in _NC_CACHE:
        _NC_CACHE[key] = _build()
    return _NC_CACHE[key]


def _host_att(W1, b1, W2, b2, cat_enc):
    h = np.maximum(cat_enc @ W1 + b1, 0.0)
    a = h @ W2 + b2
    a = a - a.max(axis=-1, keepdims=True)
    e = np.exp(a)
    return e / e.sum(axis=-1, keepdims=True)


def kernel(image, W_emb, b_emb, W_rep, b_rep, mask_table, W1, b1, W2, b2, cat_enc):
    import ml_dtypes

    f8 = ml_dtypes.float8_e4m3fn
    bf = ml_dtypes.bfloat16

    image = np.asarray(image, np.float32)
    W_emb = np.asarray(W_emb, np.float32)
    b_emb = np.asarray(b_emb, np.float32).reshape(1, D)
    W_rep = np.asarray(W_rep, np.float32)
    b_rep = np.asarray(b_rep, np.float32)
    mask_table = np.asarray(mask_table, np.float32)
    W1 = np.asarray(W1, np.float32)
    b1 = np.asarray(b1, np.float32).reshape(1, N)
    W2 = np.asarray(W2, np.float32)
    b2 = np.asarray(b2, np.float32).reshape(1, N)
    cat_enc = np.asarray(cat_enc, np.float32)

    # fold the mask (and fp8 scale) into the per-condition weights
    wm = W_rep * mask_table[:, None, :]          # [N, D, D] premasked
    if WSCALE * ESCALE != 1.0:
        wm = wm * (WSCALE * ESCALE)
    wrep_pad = np.zeros((NPAD, D, D), np.float32)
    wrep_pad[:N] = wm
    wrep_cast = wrep_pad.astype(f8 if WNAME == "fp8" else bf)

    # host-side layouts for the x GEMM
    img_t = np.ascontiguousarray(
        image.T.reshape(KF, 128, 128).transpose(1, 0, 2)
    ).astype(bf)                                  # [128, KF, 128]
    wemb_t = np.ascontiguousarray(
        W_emb.reshape(KF, 128, D).transpose(1, 0, 2)
    ).astype(bf)                                  # [128, KF, D]

    # R row r = R_OFF[g] + src*gs + gi holds condition 9*src + N_OFF[g] + gi
    GS = list(GROUPS)
    N_OFF = [sum(GS[:g]) for g in range(len(GS))]
    R_OFF = [NCORES * o for o in N_OFF]
    n_of_r = np.empty(NPAD, np.int64)
    for g in range(len(GS)):
        for src in range(NCORES):
            for gi in range(GS[g]):
                n_of_r[R_OFF[g] + src * GS[g] + gi] = NL * src + N_OFF[g] + gi
    psel = np.zeros((N, NPAD), np.float32)
    for r in range(NPAD):
        if n_of_r[r] < N:
            psel[n_of_r[r], r] = 1.0 / (WSCALE * ESCALE)

    nc = _get_nc()
    in_maps = []
    for i in range(NCORES):
        in_maps.append({
            "img_t": img_t,
            "wemb_t": wemb_t,
            "b_emb": b_emb.astype(bf),
            "w_rep_l": np.ascontiguousarray(wrep_cast[i * NL : (i + 1) * NL]),
            "w1": W1, "b1": b1, "w2": W2, "b2": b2,
            "cat_enc": cat_enc, "perm_sel": psel,
        })

    global _LAST_IN_MAPS
    _LAST_IN_MAPS = in_maps
    res = run_bass_kernel_spmd(nc, in_maps, list(range(NCORES)))

    out = np.empty((B, P + N, D), np.float32)
    for i in range(NCORES):
        oc = np.asarray(res.results[i]["out_cond"]).reshape(P, BL, D)
        out[i * BL : (i + 1) * BL, :P] = oc.transpose(1, 0, 2).astype(np.float32)
    x = np.asarray(res.results[0]["x_out"])
    out[:, P:] = x[:, None, :]

    # host correction for the (mask * b_rep) bias term (zero in practice)
    if np.any(b_rep):
        att = _host_att(W1, b1, W2, b2, cat_enc)
        corr = att @ (mask_table * b_rep)        # [P, D]
        out[:, :P] += corr[None]
    return out
